# revision 69
# baseline (speedup 1.0000x reference)
"""AFNO layer Trainium2 kernel — data-parallel over the 16 (b,t) pairs, 2 per core.

Pipeline per (b,t), per core (all matmuls bf16, accumulate f32):
  LN1 (token-major, batched stats) -> fwd DFT to 288 kept modes (matmul vs
  precomputed cos/sin, output channel-major) -> block-diag complex mixing
  (packed 128x128 matmuls, gelu / softshrink epilogues) -> PE transpose ->
  inverse DFT (matmul, token-major) -> +h +x residual -> LN2 -> PE transpose
  -> MLP (768->3072 gelu ->768) -> +res2.

Host-side folds: ln1_g into w1 (per-block diag), ln1_b vanishes in kept modes
(kx=5..28 excludes 0), ln2_g/ln2_b into mw1/mb1. All constants are
host-transposed into single contiguous SBUF images (one DMA each, issued on
gpsimd so the sync queue serves the activations first).
"""

import numpy as np
import ml_dtypes

B, T, NX, NY, E, BS = 2, 8, 32, 32, 768, 64
NB = E // BS
YM = NY // 2 + 1
KM = 12
LAM = 0.01
MODES = 24 * KM          # 288 kept modes
NTOK = NX * NY           # 1024 tokens per (b,t)
BT_PER_CORE = 2
NCORES = 8
TOK_CORE = BT_PER_CORE * NTOK   # 2048
H4 = 4 * E               # 3072
EPS = 1e-5

_BF16 = ml_dtypes.bfloat16
_FP8 = ml_dtypes.float8_e4m3
S1 = 64.0   # host scale on mw1 (fp8 needs values ~1; folded out in gelu scale)
S2 = 64.0   # host scale on mw2 (folded out in the residual-add epilogue)

_CACHE = {}


def _install_trace_shim():
    """Best-effort: register the axon NTFF profiling hook so BASS_TRACE=1 works."""
    try:
        import types, sys
        if 'antenv.axon_hooks' in sys.modules:
            return
        import antenv  # noqa
        from trn_agent_boot.trn_boot import _ntff_profile_via_ctypes
        mod = types.ModuleType('antenv.axon_hooks')
        hook = _ntff_profile_via_ctypes('/opt/axon/libaxon_pjrt.so')
        mod.get_axon_ntff_profile_hook = lambda: hook
        mod.set_axon_ntff_profile_hook = lambda h: None
        sys.modules['antenv.axon_hooks'] = mod
        from concourse import bass_utils
        bass_utils.upload_artifacts = lambda tmpdir: tmpdir
    except Exception:
        pass


def _dft_matrices():
    """ArT (1024,288), AiT, BrT (288,1024), BiT as float32 (analytic, ortho norm)."""
    xx, yy = np.meshgrid(np.arange(NX), np.arange(NY), indexing='ij')
    sx = xx.ravel().astype(np.float64)
    sy = yy.ravel().astype(np.float64)
    kxs = np.arange(YM - KM, YM + KM, dtype=np.float64)   # 5..28
    kys = np.arange(KM, dtype=np.float64)                  # 0..11
    KX, KY = np.meshgrid(kxs, kys, indexing='ij')
    mkx = KX.ravel()
    mky = KY.ravel()
    ph = 2 * np.pi * (np.outer(sx, mkx) + np.outer(sy, mky)) / 32.0  # (1024,288)
    art = np.cos(ph) / 32.0
    ait = -np.sin(ph) / 32.0
    wk = np.where(mky == 0, 1.0, 2.0)
    brt = ((np.cos(ph) / 32.0) * wk).T.copy()   # (288,1024)
    bit = ((-np.sin(ph) / 32.0) * wk).T.copy()
    return (art.astype(np.float32), ait.astype(np.float32),
            brt.astype(np.float32), bit.astype(np.float32))


def _pack_blockdiag(w):
    """w: (NB,64,64) -> (6,128,128) pairs of blocks on the diagonal."""
    out = np.zeros((NB // 2, 2 * BS, 2 * BS), np.float32)
    for j in range(NB // 2):
        out[j, :BS, :BS] = w[2 * j]
        out[j, BS:, BS:] = w[2 * j + 1]
    return out


def _sb_image(a, p):
    """(n*p, f) -> (p, n*f): partition-major SBUF image for one big DMA."""
    n = a.shape[0] // p
    return np.ascontiguousarray(
        a.reshape(n, p, a.shape[1]).transpose(1, 0, 2).reshape(p, n * a.shape[1]))


def _build_program(ln1_trivial, mb2_zero, stage=5):
    import concourse.bass as bass
    import concourse.bacc as bacc
    import concourse.mybir as mybir
    from concourse import tile
    from concourse.tile import add_dep_helper

    f32 = mybir.dt.float32
    bf16 = mybir.dt.bfloat16
    fp8 = mybir.dt.float8e4
    DR = mybir.MatmulPerfMode.DoubleRow
    AF = mybir.ActivationFunctionType
    ALU = mybir.AluOpType
    AX = mybir.AxisListType

    nc = bacc.Bacc("TRN2", target_bir_lowering=False, debug=False)

    dp = nc.declare_dram_parameter
    # x/out are host-transposed to partition-major [128, 16*E] so each DMA
    # descriptor covers a long contiguous per-partition row
    x_d = dp("x", [128, 16 * E], bf16, isOutput=False)
    art_d = dp("art", [128, 8 * MODES], fp8, isOutput=False)
    ait_d = dp("ait", [128, 8 * MODES], fp8, isOutput=False)
    bcp_d = dp("bcp", [128, 4 * NTOK], fp8, isOutput=False)
    brem_d = dp("brem", [128, 2 * NTOK], fp8, isOutput=False)
    wmix_d = dp("wmix", [128, 6 * 128 * 6], bf16, isOutput=False)  # 6 packed mats
    bmix_d = dp("bmix", [128, 36], f32, isOutput=False)  # b1r b1i b2rm b2rn b2im b2in
    mw1_d = dp("mw1f", [128, 6 * H4], fp8, isOutput=False)
    mb1_d = dp("mb1f", [128, 24], f32, isOutput=False)
    mw2_d = dp("mw2", [128, 24 * E], fp8, isOutput=False)
    ident_d = dp("ident", [128, 128], bf16, isOutput=False)
    if not ln1_trivial:
        g1r_d = dp("g1rep", [128, E], f32, isOutput=False)
        b1lr_d = dp("b1rep", [128, E], f32, isOutput=False)
    if not mb2_zero:
        mb2r_d = dp("mb2rep", [128, E], f32, isOutput=False)
    out_d = dp("out", [128, 16 * E], bf16, isOutput=True)

    with tile.TileContext(nc) as tc:
        with (
            tc.tile_pool(name="pc", bufs=1) as pc,
            tc.tile_pool(name="p4", bufs=3) as p4,
            tc.tile_pool(name="p2", bufs=2) as p2,
            tc.tile_pool(name="p8", bufs=8) as p8,
            tc.tile_pool(name="pfq", bufs=28) as pfq,
            tc.tile_pool(name="po2", bufs=3) as po2,
            tc.tile_pool(name="phid", bufs=24) as phid,
            tc.tile_pool(name="px2", bufs=6) as px2,
            tc.tile_pool(name="pst", bufs=4) as pst,
            tc.tile_pool(name="psum", bufs=2, space="PSUM") as pp,
        ):
            # ---- constants: DFT matrices + ident early (needed by phase B);
            # everything else deferred past the startup DMA burst ----
            art_all = pc.tile([128, 8 * MODES], fp8, tag="art")
            nc.gpsimd.dma_start(art_all[:], art_d[:])
            ait_all = pc.tile([128, 8 * MODES], fp8, tag="ait")
            nc.gpsimd.dma_start(ait_all[:], ait_d[:])
            ident_t = pc.tile([128, 128], bf16, tag="ident")
            nc.gpsimd.dma_start(ident_t[:], ident_d[:])
            wmix_all = pc.tile([128, 6 * 128 * 6], bf16, tag="wmix")
            bmix_all = pc.tile([128, 36], f32, tag="bmix")
            bcp_all = pc.tile([128, 4 * NTOK], fp8, tag="bcp")
            brem_all = pc.tile([128, 2 * NTOK], fp8, tag="brem")
            mw1_all = pc.tile([128, 6 * H4], fp8, tag="mw1")
            mw2_all = pc.tile([128, 24 * E], fp8, tag="mw2")
            mb1_all = pc.tile([128, 24], f32, tag="mb1")
            eps_t = pc.tile([128, 1], f32, tag="epsc")
            nc.vector.memset(eps_t[:], EPS)
            # warm the scalar-engine activation tables (Sqrt/Gelu/Relu) during
            # the input DMA so the first LN1 Sqrt isn't gated by a table load
            warm = pc.tile([128, 1], f32, tag="warm")
            nc.scalar.activation(warm[:], eps_t[:], AF.Sqrt, bias=eps_t[:])
            nc.scalar.activation(warm[:], eps_t[:], AF.Gelu, bias=eps_t[:])
            nc.scalar.activation(warm[:], eps_t[:], AF.Relu, bias=eps_t[:])
            nc.scalar.activation(warm[:], eps_t[:], AF.Identity, bias=eps_t[:])
            if not ln1_trivial:
                g1rep_t = pc.tile([128, E], f32, tag="g1rep")
                nc.gpsimd.dma_start(g1rep_t[:], g1r_d[:])
                b1rep_t = pc.tile([128, E], f32, tag="b1rep")
                nc.gpsimd.dma_start(b1rep_t[:], b1lr_d[:])
            if not mb2_zero:
                mb2rep_t = pc.tile([128, E], f32, tag="mb2rep")
                nc.gpsimd.dma_start(mb2rep_t[:], mb2r_d[:])

            artv = art_all[:].rearrange("p (q k m) -> p q k m", q=4, k=2)
            aitv = ait_all[:].rearrange("p (q k m) -> p q k m", q=4, k=2)
            bcpv = bcp_all[:].rearrange("p (c k t) -> p c k t", c=2, k=2)
            bremv = brem_all[:].rearrange("p (k t) -> p k t", k=2)
            # wmix order: w1r w1i w1in w2r w2i w2in, each (128, 6*128)
            def wm(idx, j):
                o = idx * 6 * 128 + j * 128
                return wmix_all[:, o:o + 128]
            def bm(idx, j):
                return bmix_all[:, idx * 6 + j: idx * 6 + j + 1]
            mw1v = mw1_all[:].rearrange("p (q k f) -> p q k f", q=3, k=2)
            mw2v = mw2_all[:].rearrange("p (q k e) -> p q k e", q=12, k=2)
            def mw1_c(q, fj):
                return mw1v[:, q, :, fj * 128:(fj + 1) * 128]
            def mw2_c(q, n):
                return mw2v[:, q, :, n * 384:(n + 1) * 384]
            def mb1_c(fj):
                return mb1_all[:, fj:fj + 1]

            def layernorm(src_aps, dst_pool, dst_tag, bt=0, nameprefix=None,
                          pair_fp8=False):
                """LN over 8 (128,E) APs via bn_stats -> normalized tiles.
                pair_fp8: write into 4 [128,2,E] fp8 pair tiles (DoubleRow layout).
                Returns (outs, rstds, nmrs) so hx emission can be deferred."""
                outs = []
                rstds = []
                nmrs = []
                for i in range(8):
                    xt = src_aps[i]
                    xr = xt.rearrange("p (n f) -> p n f", f=256)
                    stats = pst.tile([128, 3, 6], f32, tag="bst")
                    for s3 in range(3):
                        nc.vector.bn_stats(stats[:, s3, :], xr[:, s3, :])
                    mv = pst.tile([128, 2], f32, tag="mv")
                    nc.vector.bn_aggr(mv[:], stats[:])
                    std = pst.tile([128, 1], f32, tag="std")
                    nc.scalar.activation(std[:], mv[:, 1:2], AF.Sqrt, bias=eps_t[:])
                    rstd = pst.tile([128, 1], f32, tag="rstd", bufs=10)
                    nc.vector.reciprocal(rstd[:], std[:])
                    nmr = pst.tile([128, 1], f32, tag="nmr", bufs=10)
                    nc.vector.scalar_tensor_tensor(nmr[:], mv[:, 0:1], -1.0, rstd[:],
                                                   op0=ALU.mult, op1=ALU.mult)
                    rstds.append(rstd)
                    nmrs.append(nmr)
                    if pair_fp8:
                        if i % 2 == 0:
                            hpt = dst_pool.tile([128, 2, E], fp8, tag="hb", bufs=16,
                                                name=f"{nameprefix or dst_tag}_{bt}_{i // 2}")
                            outs.append(hpt)
                        dst_ap = outs[i // 2][:, i % 2, :]
                    else:
                        hb = dst_pool.tile([128, E], bf16, tag="hb", bufs=16,
                                           name=f"{nameprefix or dst_tag}_{bt}_{i}")
                        outs.append(hb)
                        dst_ap = hb[:]
                    if i % 3 == 2:
                        nc.scalar.activation(dst_ap, xt, AF.Identity,
                                             bias=nmr[:], scale=rstd[:])
                    else:
                        weng = nc.gpsimd if i % 3 == 0 else nc.vector
                        weng.tensor_scalar(dst_ap, xt, rstd[:], nmr[:],
                                           op0=ALU.mult, op1=ALU.add)
                return outs, rstds, nmrs

            def emit_hx(bt):
                """hx = h + x = x*(rstd+1) + nmr, off the critical path on gpsimd
                (only needed by phase D)."""
                xts = st[bt]['xts']
                rstds = st[bt]['rstds']
                nmrs = st[bt]['nmrs']
                hxs = []
                for i in range(8):
                    hxt = p8.tile([128, E], bf16, tag="hx", bufs=16,
                                  name=f"hx_{bt}_{i}")
                    if ln1_trivial:
                        r1p = pst.tile([128, 1], f32, tag="r1p", bufs=10)
                        nc.gpsimd.tensor_scalar_add(r1p[:], rstds[i][:], 1.0)
                        nc.gpsimd.tensor_scalar(hxt[:], xts[i], r1p[:], nmrs[i][:],
                                                op0=ALU.mult, op1=ALU.add)
                    else:
                        tmp = p2.tile([128, E], f32, tag="lngtmp")
                        nc.gpsimd.tensor_scalar(tmp[:], xts[i], rstds[i][:],
                                                nmrs[i][:], op0=ALU.mult, op1=ALU.add)
                        nc.gpsimd.tensor_tensor(tmp[:], tmp[:], g1rep_t[:],
                                                op=ALU.mult)
                        nc.gpsimd.tensor_tensor(tmp[:], tmp[:], b1rep_t[:],
                                                op=ALU.add)
                        nc.gpsimd.tensor_tensor(hxt[:], tmp[:], xts[i],
                                                op=ALU.add)
                    hxs.append(hxt)
                st[bt]['hx'] = hxs

            # ---- phase-interleaved pipeline over the two (b,t) shards: issue
            # order A0 A1 B0 B1 C0 C1 D0 E0 D1 E1 F00 F10 F01 F11 so one
            # shard's matmuls cover the other's LN/epilogue latency bubbles
            st = [dict() for _ in range(BT_PER_CORE)]

            def phase_A(bt):
                # x arrives host-transposed: chunk k of 128 tokens lives at
                # x_d[:, (bt*8+k)*E:(bt*8+k+1)*E]; 8 outstanding DMAs so the
                # descriptor chains fan out across DMA engines
                xts = []
                for i in range(8):
                    t = p8.tile([128, E], bf16, tag="xin", bufs=16,
                                name=f"x_{bt}_{i}")
                    eng = nc.sync if i % 2 == 0 else nc.scalar
                    c0 = (bt * 8 + i) * E
                    eng.dma_start(t[:], x_d[:, c0:c0 + E])
                    xts.append(t[:])
                hbf, rstds, nmrs = layernorm(xts, p8, "hb", bt=bt, pair_fp8=True)
                st[bt]['xts'] = xts
                st[bt]['hbf'] = hbf
                st[bt]['rstds'] = rstds
                st[bt]['nmrs'] = nmrs

            def phase_B(bt):
                # fwd DFT: FR/FI channel-major (e-chunk 128, 288), fp8 DoubleRow
                # over token pairs; psum holds 32*fr (art stored unscaled cos,
                # the /32 folded into w1 on host)
                hbf = st[bt]['hbf']
                frb = []
                fib = []
                last = None
                for j in range(6):
                    pfr = pp.tile([128, MODES], f32, tag="mmA", bufs=2)
                    for q in range(4):
                        nc.tensor.matmul(pfr[:], hbf[q][:, :, j * 128:(j + 1) * 128],
                                         artv[:, q, :, :], start=(q == 0), stop=(q == 3),
                                         perf_mode=DR)
                    fr = pfq.tile([128, MODES], bf16, tag="fq", name=f"fr{bt}_{j}")
                    nc.scalar.activation(fr[:], pfr[:], AF.Copy)
                    frb.append(fr)
                    pfi = pp.tile([128, MODES], f32, tag="mmA", bufs=2)
                    for q in range(4):
                        nc.tensor.matmul(pfi[:], hbf[q][:, :, j * 128:(j + 1) * 128],
                                         aitv[:, q, :, :], start=(q == 0), stop=(q == 3),
                                         perf_mode=DR)
                    fi = pfq.tile([128, MODES], bf16, tag="fq", name=f"fi{bt}_{j}")
                    last = nc.scalar.activation(fi[:], pfi[:], AF.Copy)
                    fib.append(fi)
                st[bt]['frb'] = frb
                st[bt]['fib'] = fib
                return last

            def phase_C(bt):
                # mixing layer 1 (complex, gelu), layer 2 (+softshrink), then
                # transposes — three software-pipelined sub-loops so the PE
                # never waits on a same-j scalar epilogue. Shrunk output goes
                # mode-major for the DoubleRow iDFT: two [128,2,E] fp8 pair
                # tiles (slot0=real slot1=imag) + a zero-padded [128,2,E] fp8
                # remainder (modes 256:288 of r in parts 0:32, i in 32:64)
                frb = st[bt]['frb']
                fib = st[bt]['fib']
                o2p = [po2.tile([128, 2, E], fp8, tag="o2p", bufs=4,
                                name=f"o2p{bt}_{c}") for c in range(2)]
                o2rem = po2.tile([128, 2, E], fp8, tag="o2rem", bufs=2,
                                 name=f"o2rem{bt}")
                nc.gpsimd.memset(o2rem[:], 0.0)
                o1rs, o1is, srs, sis = [], [], [], []
                for j in range(6):
                    p1r = pp.tile([128, MODES], f32, tag="mmA", bufs=2)
                    nc.tensor.matmul(p1r[:], wm(0, j), frb[j][:], start=True, stop=False)
                    nc.tensor.matmul(p1r[:], wm(2, j), fib[j][:], start=False, stop=True)
                    o1r = pfq.tile([128, MODES], bf16, tag="fq", name=f"o1r{bt}_{j}")
                    nc.scalar.activation(o1r[:], p1r[:], AF.Gelu, bias=bm(0, j))
                    o1rs.append(o1r)
                    p1i = pp.tile([128, MODES], f32, tag="mmA", bufs=2)
                    nc.tensor.matmul(p1i[:], wm(1, j), frb[j][:], start=True, stop=False)
                    nc.tensor.matmul(p1i[:], wm(0, j), fib[j][:], start=False, stop=True)
                    o1i = pfq.tile([128, MODES], bf16, tag="fq", name=f"o1i{bt}_{j}")
                    nc.scalar.activation(o1i[:], p1i[:], AF.Gelu, bias=bm(1, j))
                    o1is.append(o1i)
                for j in range(6):
                    o1r, o1i = o1rs[j], o1is[j]
                    p2r = pp.tile([128, MODES], f32, tag="mmA", bufs=2)
                    nc.tensor.matmul(p2r[:], wm(3, j), o1r[:], start=True, stop=False)
                    nc.tensor.matmul(p2r[:], wm(5, j), o1i[:], start=False, stop=True)
                    t1 = p2.tile([128, MODES], bf16, tag="t1")
                    t2 = p2.tile([128, MODES], bf16, tag="t2")
                    nc.scalar.activation(t1[:], p2r[:], AF.Relu, bias=bm(2, j), scale=32.0)
                    nc.scalar.activation(t2[:], p2r[:], AF.Relu, bias=bm(3, j), scale=-32.0)
                    sr = pfq.tile([128, MODES], bf16, tag="fq", name=f"shr{bt}_{j}")
                    nc.gpsimd.tensor_sub(sr[:], t1[:], t2[:])
                    srs.append(sr)
                    p2i = pp.tile([128, MODES], f32, tag="mmA", bufs=2)
                    nc.tensor.matmul(p2i[:], wm(4, j), o1r[:], start=True, stop=False)
                    nc.tensor.matmul(p2i[:], wm(3, j), o1i[:], start=False, stop=True)
                    t3 = p2.tile([128, MODES], bf16, tag="t1")
                    t4 = p2.tile([128, MODES], bf16, tag="t2")
                    nc.scalar.activation(t3[:], p2i[:], AF.Relu, bias=bm(4, j), scale=32.0)
                    nc.scalar.activation(t4[:], p2i[:], AF.Relu, bias=bm(5, j), scale=-32.0)
                    si = pfq.tile([128, MODES], bf16, tag="fq", name=f"shi{bt}_{j}")
                    nc.gpsimd.tensor_sub(si[:], t3[:], t4[:])
                    sis.append(si)
                for j in range(6):
                    sr, si = srs[j], sis[j]
                    ceng = nc.vector
                    for c in range(2):
                        ptr = pp.tile([128, 128], bf16, tag="tpm")
                        nc.tensor.transpose(ptr[:], sr[:, c * 128:(c + 1) * 128], ident_t[:])
                        ceng.tensor_copy(o2p[c][:, 0, j * 128:(j + 1) * 128], ptr[:])
                        pti = pp.tile([128, 128], bf16, tag="tpm")
                        nc.tensor.transpose(pti[:], si[:, c * 128:(c + 1) * 128], ident_t[:])
                        ceng.tensor_copy(o2p[c][:, 1, j * 128:(j + 1) * 128], pti[:])
                    ptr = pp.tile([128, 128], bf16, tag="tpm")
                    nc.tensor.transpose(ptr[0:32, :], sr[:, 256:288], ident_t[:])
                    ceng.tensor_copy(o2rem[0:32, 0, j * 128:(j + 1) * 128], ptr[0:32, :])
                    pti = pp.tile([128, 128], bf16, tag="tpm")
                    nc.tensor.transpose(pti[0:32, :], si[:, 256:288], ident_t[:])
                    ceng.tensor_copy(o2rem[32:64, 0, j * 128:(j + 1) * 128], pti[0:32, :])
                st[bt]['o2p'] = o2p
                st[bt]['o2rem'] = o2rem

            def phase_D(bt):
                # inverse DFT + residual, in place: out1 = hx += spat
                # psum = 1024*spat (32x in bcp/brem, 32x in the shrunk modes)
                o2p = st[bt]['o2p']
                o2rem = st[bt]['o2rem']
                hx = st[bt]['hx']
                for p in range(8):
                    for n in range(2):
                        ps = pp.tile([128, 384], f32, tag="big", bufs=4)
                        for c in range(2):
                            nc.tensor.matmul(ps[:], bcpv[:, c, :, p * 128:(p + 1) * 128],
                                             o2p[c][:, :, n * 384:(n + 1) * 384],
                                             start=(c == 0), stop=False, perf_mode=DR)
                        nc.tensor.matmul(ps[:], bremv[:, :, p * 128:(p + 1) * 128],
                                         o2rem[:, :, n * 384:(n + 1) * 384],
                                         start=False, stop=True, perf_mode=DR)
                        nc.vector.scalar_tensor_tensor(
                            hx[p][:, n * 384:(n + 1) * 384], ps[:], 1.0 / 1024.0,
                            hx[p][:, n * 384:(n + 1) * 384], op0=ALU.mult, op1=ALU.add)
                st[bt]['out1'] = hx

            def phase_E(bt):
                # LN2 -> h2 (normalized token-major bf16; affine folded into mw1/mb1)
                h2bf, _, _ = layernorm([t[:] for t in st[bt]['out1']], p8, "h2",
                                       bt=bt, nameprefix="h2")
                st[bt]['h2'] = h2bf

            def phase_F(bt, h):
                # MLP half: transpose h2 -> fp8 channel-major pairs, fp8 DoubleRow
                # 768->3072 gelu ->768, + res2, one 4-chunk DMA out
                h2bf = st[bt]['h2']
                out1 = st[bt]['out1']
                x2h = [px2.tile([128, 2, 512], fp8, tag="x2q", bufs=6,
                                name=f"x2h{bt}_{h}_{q}") for q in range(3)]
                for tcn in range(4):
                    p = h * 4 + tcn
                    for j in range(6):
                        pt = pp.tile([128, 128], bf16, tag="tpm")
                        nc.tensor.transpose(pt[:], h2bf[p][:, j * 128:(j + 1) * 128],
                                            ident_t[:])
                        nc.vector.tensor_copy(
                            x2h[j // 2][:, j % 2, tcn * 128:(tcn + 1) * 128], pt[:])
                hid = [phid.tile([128, 2, 512], fp8, tag="hid", bufs=24,
                                 name=f"hid{bt}_{h}_{qq}") for qq in range(12)]
                for fj in range(24):
                    ph = pp.tile([128, 512], f32, tag="big", bufs=4)
                    for q in range(3):
                        nc.tensor.matmul(ph[:], mw1_c(q, fj), x2h[q][:],
                                         start=(q == 0), stop=(q == 2),
                                         perf_mode=DR)
                    nc.scalar.activation(hid[fj // 2][:, fj % 2, :], ph[:],
                                         AF.Gelu, bias=mb1_c(fj), scale=1.0 / S1)
                for tcn in range(4):
                    p = h * 4 + tcn
                    ost = p8.tile([128, E], bf16, tag="xin", bufs=16,
                                  name=f"ost{bt}_{h}_{tcn}")
                    for n in range(2):
                        po = pp.tile([128, 384], f32, tag="big", bufs=4)
                        for qq in range(12):
                            nc.tensor.matmul(po[:],
                                             hid[qq][:, :, tcn * 128:(tcn + 1) * 128],
                                             mw2_c(qq, n),
                                             start=(qq == 0), stop=(qq == 11),
                                             perf_mode=DR)
                        nc.vector.scalar_tensor_tensor(
                            ost[:, n * 384:(n + 1) * 384], po[:], 1.0 / S2,
                            out1[p][:, n * 384:(n + 1) * 384],
                            op0=ALU.mult, op1=ALU.add)
                    if not mb2_zero:
                        nc.vector.tensor_add(ost[:], ost[:], mb2rep_t[:])
                    c0 = (bt * 8 + p) * E
                    oeng = (nc.sync, nc.scalar, nc.gpsimd)[p % 3]
                    oeng.dma_start(out_d[:, c0:c0 + E], ost[:])

            phase_A(0)
            phase_A(1)
            fi_copy = phase_B(0)
            # deferred weight loads: don't let these race the startup burst
            # (x tiles + DFT matrices) on the HBM wire; wmix/bmix first (needed
            # by phase C right after B1)
            for dd_d, dd_t in ((wmix_d, wmix_all), (bmix_d, bmix_all),
                               (bcp_d, bcp_all), (brem_d, brem_all),
                               (mw1_d, mw1_all), (mw2_d, mw2_all),
                               (mb1_d, mb1_all)):
                dd = nc.gpsimd.dma_start(dd_t[:], dd_d[:])
                add_dep_helper(dd.ins, fi_copy.ins,
                               reason="defer bulk weight DMA past fwd DFT")
            phase_B(1)
            emit_hx(0)
            emit_hx(1)
            phase_C(0)
            phase_C(1)
            phase_D(0)
            phase_E(0)
            phase_D(1)
            phase_E(1)
            phase_F(0, 0)
            phase_F(1, 0)
            phase_F(0, 1)
            phase_F(1, 1)

    nc.compile()
    return nc


LAST_EXEC_NS = None


def make_consts(w1, b1, w2, b2, ln1_g, ln1_b, ln2_g, ln2_b,
                mw1, mb1, mw2, mb2, ln1_trivial, mb2_zero):
    art, ait, brt, bit = _dft_matrices()

    # fold ln1_g into w1 (left-diag per block over the i axis)
    g_blocks = ln1_g.reshape(NB, BS)
    W1R = _pack_blockdiag(w1[0] * g_blocks[:, :, None])
    W1I = _pack_blockdiag(w1[1] * g_blocks[:, :, None])
    W2R = _pack_blockdiag(w2[0])
    W2I = _pack_blockdiag(w2[1])

    b1r = b1[0].reshape(E)
    b1i = b1[1].reshape(E)
    b2r = b2[0].reshape(E)
    b2i = b2[1].reshape(E)

    mw1f = mw1 * ln2_g[:, None]
    mb1f = (mb1 + ln2_b @ mw1).reshape(H4)

    def bf(a):
        return np.ascontiguousarray(a.astype(_BF16))

    def fp8_pairs(a, scale):
        """(2q*128, F) -> (128, q*2*F) k-pair-interleaved fp8 image for DoubleRow."""
        nq = a.shape[0] // 256
        img = (a * scale).reshape(nq, 2, 128, a.shape[1]) \
            .transpose(2, 0, 1, 3).reshape(128, 2 * nq * a.shape[1])
        return np.ascontiguousarray(img.astype(_FP8))

    # wmix image: (128, 6 mats * 6 blocks * 128), order w1r w1i w1in w2r w2i w2in
    # w1 carries the 1/32 that was removed from the fp8 DFT matrices
    mats = [W1R / 32.0, W1I / 32.0, -W1I / 32.0, W2R, W2I, -W2I]
    wmix = np.concatenate(
        [m.transpose(1, 0, 2).reshape(128, 6 * 128) for m in mats], axis=1)
    # bmix image: (128, 36): 6 vectors x 6 chunks; shrink biases carry the
    # 32x fp8-friendly scale on the shrunk modes (undone by 1/1024 after iDFT)
    bvecs = [b1r, b1i, 32.0 * (b2r - LAM), 32.0 * (-b2r - LAM),
             32.0 * (b2i - LAM), 32.0 * (-b2i - LAM)]
    bmix = np.concatenate([v.reshape(6, 128).T for v in bvecs], axis=1)

    # iDFT pair image: [128, c(2), k(2), 1024] slot0=brt, slot1=bit (32x scale)
    brt32 = brt * 32.0
    bit32 = bit * 32.0
    bcp = np.zeros((128, 2, 2, NTOK), np.float32)
    for c in range(2):
        bcp[:, c, 0, :] = brt32[c * 128:(c + 1) * 128]
        bcp[:, c, 1, :] = bit32[c * 128:(c + 1) * 128]
    # remainder modes 256:288 as a zero-padded DoubleRow pair (r parts 0:32,
    # i parts 32:64, slot1 all zero)
    brem = np.zeros((128, 2, NTOK), np.float32)
    brem[0:32, 0, :] = brt32[256:288]
    brem[32:64, 0, :] = bit32[256:288]

    consts = {
        "art": fp8_pairs(art, 32.0), "ait": fp8_pairs(ait, 32.0),
        "bcp": np.ascontiguousarray(bcp.reshape(128, 4 * NTOK).astype(_FP8)),
        "brem": np.ascontiguousarray(brem.reshape(128, 2 * NTOK).astype(_FP8)),
        "wmix": bf(wmix), "bmix": np.ascontiguousarray(bmix, np.float32),
        "mw1f": fp8_pairs(mw1f, S1),
        "mb1f": np.ascontiguousarray(mb1f.reshape(24, 128).T, np.float32),
        "mw2": fp8_pairs(mw2, S2),
        "ident": bf(np.eye(128, dtype=np.float32)),
    }
    if not ln1_trivial:
        consts["g1rep"] = np.tile(ln1_g[None, :], (128, 1)).astype(np.float32)
        consts["b1rep"] = np.tile(ln1_b[None, :], (128, 1)).astype(np.float32)
    if not mb2_zero:
        consts["mb2rep"] = np.tile(mb2[None, :], (128, 1)).astype(np.float32)
    return consts


def kernel(input, w1, b1, w2, b2, ln1_g, ln1_b, ln2_g, ln2_b, mw1, mb1, mw2, mb2):
    global LAST_EXEC_NS
    _install_trace_shim()
    import os
    from concourse.bass_utils import run_bass_kernel_spmd

    input = np.asarray(input, np.float32)
    w1 = np.asarray(w1, np.float32)
    b1 = np.asarray(b1, np.float32)
    w2 = np.asarray(w2, np.float32)
    b2 = np.asarray(b2, np.float32)
    ln1_g = np.asarray(ln1_g, np.float32)
    ln1_b = np.asarray(ln1_b, np.float32)
    ln2_g = np.asarray(ln2_g, np.float32)
    ln2_b = np.asarray(ln2_b, np.float32)
    mw1 = np.asarray(mw1, np.float32)
    mb1 = np.asarray(mb1, np.float32)
    mw2 = np.asarray(mw2, np.float32)
    mb2 = np.asarray(mb2, np.float32)

    ln1_trivial = bool(np.all(ln1_g == 1.0) and np.all(ln1_b == 0.0))
    mb2_zero = bool(np.all(mb2 == 0.0))

    key = (ln1_trivial, mb2_zero)
    if key not in _CACHE:
        _CACHE[key] = _build_program(ln1_trivial, mb2_zero)
    nc = _CACHE[key]

    consts = make_consts(w1, b1, w2, b2, ln1_g, ln1_b, ln2_g, ln2_b,
                         mw1, mb1, mw2, mb2, ln1_trivial, mb2_zero)

    xs = input.reshape(B * T, NTOK, E)
    in_maps = []
    for c in range(NCORES):
        shard = xs[c * BT_PER_CORE:(c + 1) * BT_PER_CORE].reshape(TOK_CORE, E)
        # partition-major image: img[p, k*E:(k+1)*E] = x[k*128+p, :]
        img = np.ascontiguousarray(
            shard.reshape(16, 128, E).transpose(1, 0, 2).reshape(128, 16 * E)
            .astype(_BF16))
        m = {"x": img}
        m.update(consts)
        in_maps.append(m)

    trace = bool(os.environ.get("BASS_TRACE"))
    res = run_bass_kernel_spmd(nc, in_maps, core_ids=list(range(NCORES)),
                               trace=trace)
    LAST_EXEC_NS = res.exec_time_ns
    out = np.concatenate(
        [np.asarray(res.results[c]["out"]).astype(np.float32)
         .reshape(128, 16, E).transpose(1, 0, 2)
         .reshape(BT_PER_CORE, NTOK, E) for c in range(NCORES)], axis=0)
    return out.reshape(B, T, NTOK, E)



# revision 72
# speedup vs baseline: 1.0243x; 1.0243x over previous
"""AFNO layer Trainium2 kernel — data-parallel over the 16 (b,t) pairs, 2 per core.

Pipeline per (b,t), per core (all matmuls bf16, accumulate f32):
  LN1 (token-major, batched stats) -> fwd DFT to 288 kept modes (matmul vs
  precomputed cos/sin, output channel-major) -> block-diag complex mixing
  (packed 128x128 matmuls, gelu / softshrink epilogues) -> PE transpose ->
  inverse DFT (matmul, token-major) -> +h +x residual -> LN2 -> PE transpose
  -> MLP (768->3072 gelu ->768) -> +res2.

Host-side folds: ln1_g into w1 (per-block diag), ln1_b vanishes in kept modes
(kx=5..28 excludes 0), ln2_g/ln2_b into mw1/mb1. All constants are
host-transposed into single contiguous SBUF images (one DMA each, issued on
gpsimd so the sync queue serves the activations first).
"""

import numpy as np
import ml_dtypes

B, T, NX, NY, E, BS = 2, 8, 32, 32, 768, 64
NB = E // BS
YM = NY // 2 + 1
KM = 12
LAM = 0.01
MODES = 24 * KM          # 288 kept modes
NTOK = NX * NY           # 1024 tokens per (b,t)
BT_PER_CORE = 2
NCORES = 8
TOK_CORE = BT_PER_CORE * NTOK   # 2048
H4 = 4 * E               # 3072
EPS = 1e-5

_BF16 = ml_dtypes.bfloat16
_FP8 = ml_dtypes.float8_e4m3
S1 = 64.0   # host scale on mw1 (fp8 needs values ~1; folded out in gelu scale)
S2 = 64.0   # host scale on mw2 (folded out in the residual-add epilogue)

_CACHE = {}


def _install_trace_shim():
    """Best-effort: register the axon NTFF profiling hook so BASS_TRACE=1 works."""
    try:
        import types, sys
        if 'antenv.axon_hooks' in sys.modules:
            return
        import antenv  # noqa
        from trn_agent_boot.trn_boot import _ntff_profile_via_ctypes
        mod = types.ModuleType('antenv.axon_hooks')
        hook = _ntff_profile_via_ctypes('/opt/axon/libaxon_pjrt.so')
        mod.get_axon_ntff_profile_hook = lambda: hook
        mod.set_axon_ntff_profile_hook = lambda h: None
        sys.modules['antenv.axon_hooks'] = mod
        from concourse import bass_utils
        bass_utils.upload_artifacts = lambda tmpdir: tmpdir
    except Exception:
        pass


def _dft_matrices():
    """ArT (1024,288), AiT, BrT (288,1024), BiT as float32 (analytic, ortho norm)."""
    xx, yy = np.meshgrid(np.arange(NX), np.arange(NY), indexing='ij')
    sx = xx.ravel().astype(np.float64)
    sy = yy.ravel().astype(np.float64)
    kxs = np.arange(YM - KM, YM + KM, dtype=np.float64)   # 5..28
    kys = np.arange(KM, dtype=np.float64)                  # 0..11
    KX, KY = np.meshgrid(kxs, kys, indexing='ij')
    mkx = KX.ravel()
    mky = KY.ravel()
    ph = 2 * np.pi * (np.outer(sx, mkx) + np.outer(sy, mky)) / 32.0  # (1024,288)
    art = np.cos(ph) / 32.0
    ait = -np.sin(ph) / 32.0
    wk = np.where(mky == 0, 1.0, 2.0)
    brt = ((np.cos(ph) / 32.0) * wk).T.copy()   # (288,1024)
    bit = ((-np.sin(ph) / 32.0) * wk).T.copy()
    return (art.astype(np.float32), ait.astype(np.float32),
            brt.astype(np.float32), bit.astype(np.float32))


def _pack_blockdiag(w):
    """w: (NB,64,64) -> (6,128,128) pairs of blocks on the diagonal."""
    out = np.zeros((NB // 2, 2 * BS, 2 * BS), np.float32)
    for j in range(NB // 2):
        out[j, :BS, :BS] = w[2 * j]
        out[j, BS:, BS:] = w[2 * j + 1]
    return out


def _sb_image(a, p):
    """(n*p, f) -> (p, n*f): partition-major SBUF image for one big DMA."""
    n = a.shape[0] // p
    return np.ascontiguousarray(
        a.reshape(n, p, a.shape[1]).transpose(1, 0, 2).reshape(p, n * a.shape[1]))


def _build_program(ln1_trivial, mb2_zero, stage=5):
    import concourse.bass as bass
    import concourse.bacc as bacc
    import concourse.mybir as mybir
    from concourse import tile
    from concourse.tile import add_dep_helper

    f32 = mybir.dt.float32
    bf16 = mybir.dt.bfloat16
    fp8 = mybir.dt.float8e4
    DR = mybir.MatmulPerfMode.DoubleRow
    AF = mybir.ActivationFunctionType
    ALU = mybir.AluOpType
    AX = mybir.AxisListType

    nc = bacc.Bacc("TRN2", target_bir_lowering=False, debug=False)

    dp = nc.declare_dram_parameter
    # x/out are host-transposed to partition-major [128, 16*E] so each DMA
    # descriptor covers a long contiguous per-partition row
    x_d = dp("x", [128, 16 * E], bf16, isOutput=False)
    art_d = dp("art", [128, 8 * MODES], fp8, isOutput=False)
    ait_d = dp("ait", [128, 8 * MODES], fp8, isOutput=False)
    bcp_d = dp("bcp", [128, 4 * NTOK], fp8, isOutput=False)
    brem_d = dp("brem", [128, 2 * NTOK], fp8, isOutput=False)
    wmix_d = dp("wmix", [128, 6 * 128 * 6], bf16, isOutput=False)  # 6 packed mats
    bmix_d = dp("bmix", [128, 36], f32, isOutput=False)  # b1r b1i b2rm b2rn b2im b2in
    mw1_d = dp("mw1f", [128, 6 * H4], fp8, isOutput=False)
    mb1_d = dp("mb1f", [128, 24], f32, isOutput=False)
    mw2_d = dp("mw2", [128, 24 * E], fp8, isOutput=False)
    ident_d = dp("ident", [128, 128], bf16, isOutput=False)
    if not ln1_trivial:
        g1r_d = dp("g1rep", [128, E], f32, isOutput=False)
        b1lr_d = dp("b1rep", [128, E], f32, isOutput=False)
    if not mb2_zero:
        mb2r_d = dp("mb2rep", [128, E], f32, isOutput=False)
    out_d = dp("out", [128, 16 * E], bf16, isOutput=True)

    with tile.TileContext(nc) as tc:
        with (
            tc.tile_pool(name="pc", bufs=1) as pc,
            tc.tile_pool(name="p4", bufs=3) as p4,
            tc.tile_pool(name="p2", bufs=2) as p2,
            tc.tile_pool(name="p8", bufs=8) as p8,
            tc.tile_pool(name="pfq", bufs=28) as pfq,
            tc.tile_pool(name="po2", bufs=3) as po2,
            tc.tile_pool(name="phid", bufs=24) as phid,
            tc.tile_pool(name="px2", bufs=6) as px2,
            tc.tile_pool(name="pst", bufs=4) as pst,
            tc.tile_pool(name="psum", bufs=2, space="PSUM") as pp,
        ):
            # ---- constants: DFT matrices + ident early (needed by phase B);
            # everything else deferred past the startup DMA burst ----
            art_all = pc.tile([128, 8 * MODES], fp8, tag="art")
            nc.gpsimd.dma_start(art_all[:], art_d[:])
            ait_all = pc.tile([128, 8 * MODES], fp8, tag="ait")
            nc.gpsimd.dma_start(ait_all[:], ait_d[:])
            ident_t = pc.tile([128, 128], bf16, tag="ident")
            nc.gpsimd.dma_start(ident_t[:], ident_d[:])
            wmix_all = pc.tile([128, 6 * 128 * 6], bf16, tag="wmix")
            bmix_all = pc.tile([128, 36], f32, tag="bmix")
            bcp_all = pc.tile([128, 4 * NTOK], fp8, tag="bcp")
            brem_all = pc.tile([128, 2 * NTOK], fp8, tag="brem")
            mw1_all = pc.tile([128, 6 * H4], fp8, tag="mw1")
            mw2_all = pc.tile([128, 24 * E], fp8, tag="mw2")
            mb1_all = pc.tile([128, 24], f32, tag="mb1")
            eps_t = pc.tile([128, 1], f32, tag="epsc")
            nc.vector.memset(eps_t[:], EPS)
            # warm the scalar-engine activation tables (Sqrt/Gelu/Relu) during
            # the input DMA so the first LN1 Sqrt isn't gated by a table load
            warm = pc.tile([128, 1], f32, tag="warm")
            nc.scalar.activation(warm[:], eps_t[:], AF.Sqrt, bias=eps_t[:])
            nc.scalar.activation(warm[:], eps_t[:], AF.Gelu, bias=eps_t[:])
            nc.scalar.activation(warm[:], eps_t[:], AF.Relu, bias=eps_t[:])
            nc.scalar.activation(warm[:], eps_t[:], AF.Identity, bias=eps_t[:])
            if not ln1_trivial:
                g1rep_t = pc.tile([128, E], f32, tag="g1rep")
                nc.gpsimd.dma_start(g1rep_t[:], g1r_d[:])
                b1rep_t = pc.tile([128, E], f32, tag="b1rep")
                nc.gpsimd.dma_start(b1rep_t[:], b1lr_d[:])
            if not mb2_zero:
                mb2rep_t = pc.tile([128, E], f32, tag="mb2rep")
                nc.gpsimd.dma_start(mb2rep_t[:], mb2r_d[:])

            artv = art_all[:].rearrange("p (q k m) -> p q k m", q=4, k=2)
            aitv = ait_all[:].rearrange("p (q k m) -> p q k m", q=4, k=2)
            bcpv = bcp_all[:].rearrange("p (c k t) -> p c k t", c=2, k=2)
            bremv = brem_all[:].rearrange("p (k t) -> p k t", k=2)
            # wmix order: w1r w1i w1in w2r w2i w2in, each (128, 6*128)
            def wm(idx, j):
                o = idx * 6 * 128 + j * 128
                return wmix_all[:, o:o + 128]
            def bm(idx, j):
                return bmix_all[:, idx * 6 + j: idx * 6 + j + 1]
            mw1v = mw1_all[:].rearrange("p (q k f) -> p q k f", q=3, k=2)
            mw2v = mw2_all[:].rearrange("p (q k e) -> p q k e", q=12, k=2)
            def mw1_c(q, fj):
                return mw1v[:, q, :, fj * 128:(fj + 1) * 128]
            def mw2_c(q, n):
                return mw2v[:, q, :, n * 384:(n + 1) * 384]
            def mb1_c(fj):
                return mb1_all[:, fj:fj + 1]

            def layernorm(src_aps, dst_pool, dst_tag, bt=0, nameprefix=None,
                          pair_fp8=False):
                """LN over 8 (128,E) APs via bn_stats -> normalized tiles.
                pair_fp8: write into 4 [128,2,E] fp8 pair tiles (DoubleRow layout).
                Returns (outs, rstds, nmrs) so hx emission can be deferred."""
                outs = []
                rstds = []
                nmrs = []
                for i in range(8):
                    xt = src_aps[i]
                    xr = xt.rearrange("p (n f) -> p n f", f=256)
                    stats = pst.tile([128, 3, 6], f32, tag="bst")
                    for s3 in range(3):
                        nc.vector.bn_stats(stats[:, s3, :], xr[:, s3, :])
                    mv = pst.tile([128, 2], f32, tag="mv")
                    nc.vector.bn_aggr(mv[:], stats[:])
                    std = pst.tile([128, 1], f32, tag="std")
                    nc.scalar.activation(std[:], mv[:, 1:2], AF.Sqrt, bias=eps_t[:])
                    rstd = pst.tile([128, 1], f32, tag="rstd", bufs=10)
                    nc.vector.reciprocal(rstd[:], std[:])
                    nmr = pst.tile([128, 1], f32, tag="nmr", bufs=10)
                    nc.vector.scalar_tensor_tensor(nmr[:], mv[:, 0:1], -1.0, rstd[:],
                                                   op0=ALU.mult, op1=ALU.mult)
                    rstds.append(rstd)
                    nmrs.append(nmr)
                    if pair_fp8:
                        if i % 2 == 0:
                            hpt = dst_pool.tile([128, 2, E], fp8, tag="hb", bufs=16,
                                                name=f"{nameprefix or dst_tag}_{bt}_{i // 2}")
                            outs.append(hpt)
                        dst_ap = outs[i // 2][:, i % 2, :]
                    else:
                        hb = dst_pool.tile([128, E], bf16, tag="hb", bufs=16,
                                           name=f"{nameprefix or dst_tag}_{bt}_{i}")
                        outs.append(hb)
                        dst_ap = hb[:]
                    if i % 3 == 2:
                        nc.scalar.activation(dst_ap, xt, AF.Identity,
                                             bias=nmr[:], scale=rstd[:])
                    else:
                        weng = nc.gpsimd if i % 3 == 0 else nc.vector
                        weng.tensor_scalar(dst_ap, xt, rstd[:], nmr[:],
                                           op0=ALU.mult, op1=ALU.add)
                return outs, rstds, nmrs

            def emit_hx(bt):
                """hx = h + x = x*(rstd+1) + nmr, off the critical path on gpsimd
                (only needed by phase D)."""
                xts = st[bt]['xts']
                rstds = st[bt]['rstds']
                nmrs = st[bt]['nmrs']
                hxs = []
                for i in range(8):
                    hxt = p8.tile([128, E], bf16, tag="hx", bufs=16,
                                  name=f"hx_{bt}_{i}")
                    if ln1_trivial:
                        r1p = pst.tile([128, 1], f32, tag="r1p", bufs=10)
                        nc.gpsimd.tensor_scalar_add(r1p[:], rstds[i][:], 1.0)
                        nc.gpsimd.tensor_scalar(hxt[:], xts[i], r1p[:], nmrs[i][:],
                                                op0=ALU.mult, op1=ALU.add)
                    else:
                        tmp = p2.tile([128, E], f32, tag="lngtmp")
                        nc.gpsimd.tensor_scalar(tmp[:], xts[i], rstds[i][:],
                                                nmrs[i][:], op0=ALU.mult, op1=ALU.add)
                        nc.gpsimd.tensor_tensor(tmp[:], tmp[:], g1rep_t[:],
                                                op=ALU.mult)
                        nc.gpsimd.tensor_tensor(tmp[:], tmp[:], b1rep_t[:],
                                                op=ALU.add)
                        nc.gpsimd.tensor_tensor(hxt[:], tmp[:], xts[i],
                                                op=ALU.add)
                    hxs.append(hxt)
                st[bt]['hx'] = hxs

            # ---- phase-interleaved pipeline over the two (b,t) shards: issue
            # order A0 A1 B0 B1 C0 C1 D0 E0 D1 E1 F00 F10 F01 F11 so one
            # shard's matmuls cover the other's LN/epilogue latency bubbles
            st = [dict() for _ in range(BT_PER_CORE)]

            def phase_A(bt):
                # x arrives host-transposed: chunk k of 128 tokens lives at
                # x_d[:, (bt*8+k)*E:(bt*8+k+1)*E]; 8 outstanding DMAs so the
                # descriptor chains fan out across DMA engines
                xts = []
                for i in range(8):
                    t = p8.tile([128, E], bf16, tag="xin", bufs=16,
                                name=f"x_{bt}_{i}")
                    c0 = (bt * 8 + i) * E
                    # two half-column transfers -> two DMA engines per tile
                    nc.sync.dma_start(t[:, 0:E // 2], x_d[:, c0:c0 + E // 2])
                    nc.scalar.dma_start(t[:, E // 2:E], x_d[:, c0 + E // 2:c0 + E])
                    xts.append(t[:])
                hbf, rstds, nmrs = layernorm(xts, p8, "hb", bt=bt, pair_fp8=True)
                st[bt]['xts'] = xts
                st[bt]['hbf'] = hbf
                st[bt]['rstds'] = rstds
                st[bt]['nmrs'] = nmrs

            def phase_B(bt):
                # fwd DFT: FR/FI channel-major (e-chunk 128, 288), fp8 DoubleRow
                # over token pairs; psum holds 32*fr (art stored unscaled cos,
                # the /32 folded into w1 on host)
                hbf = st[bt]['hbf']
                frb = []
                fib = []
                last = None
                for j in range(6):
                    pfr = pp.tile([128, MODES], f32, tag="mmA", bufs=2)
                    for q in range(4):
                        nc.tensor.matmul(pfr[:], hbf[q][:, :, j * 128:(j + 1) * 128],
                                         artv[:, q, :, :], start=(q == 0), stop=(q == 3),
                                         perf_mode=DR)
                    fr = pfq.tile([128, MODES], bf16, tag="fq", name=f"fr{bt}_{j}")
                    nc.scalar.activation(fr[:], pfr[:], AF.Copy)
                    frb.append(fr)
                    pfi = pp.tile([128, MODES], f32, tag="mmA", bufs=2)
                    for q in range(4):
                        nc.tensor.matmul(pfi[:], hbf[q][:, :, j * 128:(j + 1) * 128],
                                         aitv[:, q, :, :], start=(q == 0), stop=(q == 3),
                                         perf_mode=DR)
                    fi = pfq.tile([128, MODES], bf16, tag="fq", name=f"fi{bt}_{j}")
                    last = nc.scalar.activation(fi[:], pfi[:], AF.Copy)
                    fib.append(fi)
                st[bt]['frb'] = frb
                st[bt]['fib'] = fib
                return last

            def phase_C(bt):
                # mixing layer 1 (complex, gelu), layer 2 (+softshrink), then
                # transposes — three software-pipelined sub-loops so the PE
                # never waits on a same-j scalar epilogue. Shrunk output goes
                # mode-major for the DoubleRow iDFT: two [128,2,E] fp8 pair
                # tiles (slot0=real slot1=imag) + a zero-padded [128,2,E] fp8
                # remainder (modes 256:288 of r in parts 0:32, i in 32:64)
                frb = st[bt]['frb']
                fib = st[bt]['fib']
                o2p = [po2.tile([128, 2, E], fp8, tag="o2p", bufs=4,
                                name=f"o2p{bt}_{c}") for c in range(2)]
                o2rem = po2.tile([128, 2, E], fp8, tag="o2rem", bufs=2,
                                 name=f"o2rem{bt}")
                nc.gpsimd.memset(o2rem[:], 0.0)
                o1rs, o1is, srs, sis = [], [], [], []
                for j in range(6):
                    p1r = pp.tile([128, MODES], f32, tag="mmA", bufs=2)
                    nc.tensor.matmul(p1r[:], wm(0, j), frb[j][:], start=True, stop=False)
                    nc.tensor.matmul(p1r[:], wm(2, j), fib[j][:], start=False, stop=True)
                    o1r = pfq.tile([128, MODES], bf16, tag="fq", name=f"o1r{bt}_{j}")
                    nc.scalar.activation(o1r[:], p1r[:], AF.Gelu, bias=bm(0, j))
                    o1rs.append(o1r)
                    p1i = pp.tile([128, MODES], f32, tag="mmA", bufs=2)
                    nc.tensor.matmul(p1i[:], wm(1, j), frb[j][:], start=True, stop=False)
                    nc.tensor.matmul(p1i[:], wm(0, j), fib[j][:], start=False, stop=True)
                    o1i = pfq.tile([128, MODES], bf16, tag="fq", name=f"o1i{bt}_{j}")
                    nc.scalar.activation(o1i[:], p1i[:], AF.Gelu, bias=bm(1, j))
                    o1is.append(o1i)
                for j in range(6):
                    o1r, o1i = o1rs[j], o1is[j]
                    p2r = pp.tile([128, MODES], f32, tag="mmA", bufs=2)
                    nc.tensor.matmul(p2r[:], wm(3, j), o1r[:], start=True, stop=False)
                    nc.tensor.matmul(p2r[:], wm(5, j), o1i[:], start=False, stop=True)
                    t1 = p2.tile([128, MODES], bf16, tag="t1")
                    t2 = p2.tile([128, MODES], bf16, tag="t2")
                    nc.scalar.activation(t1[:], p2r[:], AF.Relu, bias=bm(2, j), scale=32.0)
                    nc.scalar.activation(t2[:], p2r[:], AF.Relu, bias=bm(3, j), scale=-32.0)
                    sr = pfq.tile([128, MODES], bf16, tag="fq", name=f"shr{bt}_{j}")
                    nc.gpsimd.tensor_sub(sr[:], t1[:], t2[:])
                    srs.append(sr)
                    p2i = pp.tile([128, MODES], f32, tag="mmA", bufs=2)
                    nc.tensor.matmul(p2i[:], wm(4, j), o1r[:], start=True, stop=False)
                    nc.tensor.matmul(p2i[:], wm(3, j), o1i[:], start=False, stop=True)
                    t3 = p2.tile([128, MODES], bf16, tag="t1")
                    t4 = p2.tile([128, MODES], bf16, tag="t2")
                    nc.scalar.activation(t3[:], p2i[:], AF.Relu, bias=bm(4, j), scale=32.0)
                    nc.scalar.activation(t4[:], p2i[:], AF.Relu, bias=bm(5, j), scale=-32.0)
                    si = pfq.tile([128, MODES], bf16, tag="fq", name=f"shi{bt}_{j}")
                    nc.gpsimd.tensor_sub(si[:], t3[:], t4[:])
                    sis.append(si)
                for j in range(6):
                    sr, si = srs[j], sis[j]
                    ceng = nc.vector
                    for c in range(2):
                        ptr = pp.tile([128, 128], bf16, tag="tpm")
                        nc.tensor.transpose(ptr[:], sr[:, c * 128:(c + 1) * 128], ident_t[:])
                        ceng.tensor_copy(o2p[c][:, 0, j * 128:(j + 1) * 128], ptr[:])
                        pti = pp.tile([128, 128], bf16, tag="tpm")
                        nc.tensor.transpose(pti[:], si[:, c * 128:(c + 1) * 128], ident_t[:])
                        ceng.tensor_copy(o2p[c][:, 1, j * 128:(j + 1) * 128], pti[:])
                    ptr = pp.tile([128, 128], bf16, tag="tpm")
                    nc.tensor.transpose(ptr[0:32, :], sr[:, 256:288], ident_t[:])
                    ceng.tensor_copy(o2rem[0:32, 0, j * 128:(j + 1) * 128], ptr[0:32, :])
                    pti = pp.tile([128, 128], bf16, tag="tpm")
                    nc.tensor.transpose(pti[0:32, :], si[:, 256:288], ident_t[:])
                    ceng.tensor_copy(o2rem[32:64, 0, j * 128:(j + 1) * 128], pti[0:32, :])
                st[bt]['o2p'] = o2p
                st[bt]['o2rem'] = o2rem

            def phase_D(bt):
                # inverse DFT + residual, in place: out1 = hx += spat
                # psum = 1024*spat (32x in bcp/brem, 32x in the shrunk modes)
                o2p = st[bt]['o2p']
                o2rem = st[bt]['o2rem']
                hx = st[bt]['hx']
                for p in range(8):
                    for n in range(2):
                        ps = pp.tile([128, 384], f32, tag="big", bufs=4)
                        for c in range(2):
                            nc.tensor.matmul(ps[:], bcpv[:, c, :, p * 128:(p + 1) * 128],
                                             o2p[c][:, :, n * 384:(n + 1) * 384],
                                             start=(c == 0), stop=False, perf_mode=DR)
                        nc.tensor.matmul(ps[:], bremv[:, :, p * 128:(p + 1) * 128],
                                         o2rem[:, :, n * 384:(n + 1) * 384],
                                         start=False, stop=True, perf_mode=DR)
                        if n == 0:
                            nc.vector.scalar_tensor_tensor(
                                hx[p][:, 0:384], ps[:], 1.0 / 1024.0,
                                hx[p][:, 0:384], op0=ALU.mult, op1=ALU.add)
                        else:
                            # keep vector free for LN2 stats: scalar drains the
                            # psum, gpsimd folds it into hx
                            sp = p2.tile([128, 384], bf16, tag="dsp")
                            nc.scalar.activation(sp[:], ps[:], AF.Copy,
                                                 scale=1.0 / 1024.0)
                            nc.gpsimd.tensor_tensor(
                                hx[p][:, 384:768], sp[:], hx[p][:, 384:768],
                                op=ALU.add)
                st[bt]['out1'] = hx

            def phase_E(bt):
                # LN2 -> h2 (normalized token-major bf16; affine folded into mw1/mb1)
                h2bf, _, _ = layernorm([t[:] for t in st[bt]['out1']], p8, "h2",
                                       bt=bt, nameprefix="h2")
                st[bt]['h2'] = h2bf

            def phase_F(bt, h):
                # MLP half: transpose h2 -> fp8 channel-major pairs, fp8 DoubleRow
                # 768->3072 gelu ->768, + res2, one 4-chunk DMA out
                h2bf = st[bt]['h2']
                out1 = st[bt]['out1']
                x2h = [px2.tile([128, 2, 512], fp8, tag="x2q", bufs=6,
                                name=f"x2h{bt}_{h}_{q}") for q in range(3)]
                for tcn in range(4):
                    p = h * 4 + tcn
                    for j in range(6):
                        pt = pp.tile([128, 128], bf16, tag="tpm")
                        nc.tensor.transpose(pt[:], h2bf[p][:, j * 128:(j + 1) * 128],
                                            ident_t[:])
                        nc.vector.tensor_copy(
                            x2h[j // 2][:, j % 2, tcn * 128:(tcn + 1) * 128], pt[:])
                hid = [phid.tile([128, 2, 512], fp8, tag="hid", bufs=24,
                                 name=f"hid{bt}_{h}_{qq}") for qq in range(12)]
                for fj in range(24):
                    ph = pp.tile([128, 512], f32, tag="big", bufs=4)
                    for q in range(3):
                        nc.tensor.matmul(ph[:], mw1_c(q, fj), x2h[q][:],
                                         start=(q == 0), stop=(q == 2),
                                         perf_mode=DR)
                    nc.scalar.activation(hid[fj // 2][:, fj % 2, :], ph[:],
                                         AF.Gelu, bias=mb1_c(fj), scale=1.0 / S1)
                for tcn in range(4):
                    p = h * 4 + tcn
                    ost = p8.tile([128, E], bf16, tag="xin", bufs=16,
                                  name=f"ost{bt}_{h}_{tcn}")
                    for n in range(2):
                        po = pp.tile([128, 384], f32, tag="big", bufs=4)
                        for qq in range(12):
                            nc.tensor.matmul(po[:],
                                             hid[qq][:, :, tcn * 128:(tcn + 1) * 128],
                                             mw2_c(qq, n),
                                             start=(qq == 0), stop=(qq == 11),
                                             perf_mode=DR)
                        nc.vector.scalar_tensor_tensor(
                            ost[:, n * 384:(n + 1) * 384], po[:], 1.0 / S2,
                            out1[p][:, n * 384:(n + 1) * 384],
                            op0=ALU.mult, op1=ALU.add)
                    if not mb2_zero:
                        nc.vector.tensor_add(ost[:], ost[:], mb2rep_t[:])
                    c0 = (bt * 8 + p) * E
                    oeng = (nc.sync, nc.scalar, nc.gpsimd)[p % 3]
                    oeng2 = (nc.scalar, nc.gpsimd, nc.sync)[p % 3]
                    oeng.dma_start(out_d[:, c0:c0 + E // 2], ost[:, 0:E // 2])
                    oeng2.dma_start(out_d[:, c0 + E // 2:c0 + E], ost[:, E // 2:E])

            phase_A(0)
            phase_A(1)
            fi_copy = phase_B(0)
            # deferred weight loads: don't let these race the startup burst
            # (x tiles + DFT matrices) on the HBM wire; wmix/bmix first (needed
            # by phase C right after B1)
            for dd_d, dd_t in ((wmix_d, wmix_all), (bmix_d, bmix_all),
                               (bcp_d, bcp_all), (brem_d, brem_all),
                               (mw1_d, mw1_all), (mw2_d, mw2_all),
                               (mb1_d, mb1_all)):
                dd = nc.gpsimd.dma_start(dd_t[:], dd_d[:])
                add_dep_helper(dd.ins, fi_copy.ins,
                               reason="defer bulk weight DMA past fwd DFT")
            phase_B(1)
            emit_hx(0)
            emit_hx(1)
            phase_C(0)
            phase_C(1)
            phase_D(0)
            phase_E(0)
            phase_D(1)
            phase_E(1)
            phase_F(0, 0)
            phase_F(1, 0)
            phase_F(0, 1)
            phase_F(1, 1)

    nc.compile()
    return nc


LAST_EXEC_NS = None


def make_consts(w1, b1, w2, b2, ln1_g, ln1_b, ln2_g, ln2_b,
                mw1, mb1, mw2, mb2, ln1_trivial, mb2_zero):
    art, ait, brt, bit = _dft_matrices()

    # fold ln1_g into w1 (left-diag per block over the i axis)
    g_blocks = ln1_g.reshape(NB, BS)
    W1R = _pack_blockdiag(w1[0] * g_blocks[:, :, None])
    W1I = _pack_blockdiag(w1[1] * g_blocks[:, :, None])
    W2R = _pack_blockdiag(w2[0])
    W2I = _pack_blockdiag(w2[1])

    b1r = b1[0].reshape(E)
    b1i = b1[1].reshape(E)
    b2r = b2[0].reshape(E)
    b2i = b2[1].reshape(E)

    mw1f = mw1 * ln2_g[:, None]
    mb1f = (mb1 + ln2_b @ mw1).reshape(H4)

    def bf(a):
        return np.ascontiguousarray(a.astype(_BF16))

    def fp8_pairs(a, scale):
        """(2q*128, F) -> (128, q*2*F) k-pair-interleaved fp8 image for DoubleRow."""
        nq = a.shape[0] // 256
        img = (a * scale).reshape(nq, 2, 128, a.shape[1]) \
            .transpose(2, 0, 1, 3).reshape(128, 2 * nq * a.shape[1])
        return np.ascontiguousarray(img.astype(_FP8))

    # wmix image: (128, 6 mats * 6 blocks * 128), order w1r w1i w1in w2r w2i w2in
    # w1 carries the 1/32 that was removed from the fp8 DFT matrices
    mats = [W1R / 32.0, W1I / 32.0, -W1I / 32.0, W2R, W2I, -W2I]
    wmix = np.concatenate(
        [m.transpose(1, 0, 2).reshape(128, 6 * 128) for m in mats], axis=1)
    # bmix image: (128, 36): 6 vectors x 6 chunks; shrink biases carry the
    # 32x fp8-friendly scale on the shrunk modes (undone by 1/1024 after iDFT)
    bvecs = [b1r, b1i, 32.0 * (b2r - LAM), 32.0 * (-b2r - LAM),
             32.0 * (b2i - LAM), 32.0 * (-b2i - LAM)]
    bmix = np.concatenate([v.reshape(6, 128).T for v in bvecs], axis=1)

    # iDFT pair image: [128, c(2), k(2), 1024] slot0=brt, slot1=bit (32x scale)
    brt32 = brt * 32.0
    bit32 = bit * 32.0
    bcp = np.zeros((128, 2, 2, NTOK), np.float32)
    for c in range(2):
        bcp[:, c, 0, :] = brt32[c * 128:(c + 1) * 128]
        bcp[:, c, 1, :] = bit32[c * 128:(c + 1) * 128]
    # remainder modes 256:288 as a zero-padded DoubleRow pair (r parts 0:32,
    # i parts 32:64, slot1 all zero)
    brem = np.zeros((128, 2, NTOK), np.float32)
    brem[0:32, 0, :] = brt32[256:288]
    brem[32:64, 0, :] = bit32[256:288]

    consts = {
        "art": fp8_pairs(art, 32.0), "ait": fp8_pairs(ait, 32.0),
        "bcp": np.ascontiguousarray(bcp.reshape(128, 4 * NTOK).astype(_FP8)),
        "brem": np.ascontiguousarray(brem.reshape(128, 2 * NTOK).astype(_FP8)),
        "wmix": bf(wmix), "bmix": np.ascontiguousarray(bmix, np.float32),
        "mw1f": fp8_pairs(mw1f, S1),
        "mb1f": np.ascontiguousarray(mb1f.reshape(24, 128).T, np.float32),
        "mw2": fp8_pairs(mw2, S2),
        "ident": bf(np.eye(128, dtype=np.float32)),
    }
    if not ln1_trivial:
        consts["g1rep"] = np.tile(ln1_g[None, :], (128, 1)).astype(np.float32)
        consts["b1rep"] = np.tile(ln1_b[None, :], (128, 1)).astype(np.float32)
    if not mb2_zero:
        consts["mb2rep"] = np.tile(mb2[None, :], (128, 1)).astype(np.float32)
    return consts


def kernel(input, w1, b1, w2, b2, ln1_g, ln1_b, ln2_g, ln2_b, mw1, mb1, mw2, mb2):
    global LAST_EXEC_NS
    _install_trace_shim()
    import os
    from concourse.bass_utils import run_bass_kernel_spmd

    input = np.asarray(input, np.float32)
    w1 = np.asarray(w1, np.float32)
    b1 = np.asarray(b1, np.float32)
    w2 = np.asarray(w2, np.float32)
    b2 = np.asarray(b2, np.float32)
    ln1_g = np.asarray(ln1_g, np.float32)
    ln1_b = np.asarray(ln1_b, np.float32)
    ln2_g = np.asarray(ln2_g, np.float32)
    ln2_b = np.asarray(ln2_b, np.float32)
    mw1 = np.asarray(mw1, np.float32)
    mb1 = np.asarray(mb1, np.float32)
    mw2 = np.asarray(mw2, np.float32)
    mb2 = np.asarray(mb2, np.float32)

    ln1_trivial = bool(np.all(ln1_g == 1.0) and np.all(ln1_b == 0.0))
    mb2_zero = bool(np.all(mb2 == 0.0))

    key = (ln1_trivial, mb2_zero)
    if key not in _CACHE:
        _CACHE[key] = _build_program(ln1_trivial, mb2_zero)
    nc = _CACHE[key]

    consts = make_consts(w1, b1, w2, b2, ln1_g, ln1_b, ln2_g, ln2_b,
                         mw1, mb1, mw2, mb2, ln1_trivial, mb2_zero)

    xs = input.reshape(B * T, NTOK, E)
    in_maps = []
    for c in range(NCORES):
        shard = xs[c * BT_PER_CORE:(c + 1) * BT_PER_CORE].reshape(TOK_CORE, E)
        # partition-major image: img[p, k*E:(k+1)*E] = x[k*128+p, :]
        img = np.ascontiguousarray(
            shard.reshape(16, 128, E).transpose(1, 0, 2).reshape(128, 16 * E)
            .astype(_BF16))
        m = {"x": img}
        m.update(consts)
        in_maps.append(m)

    trace = bool(os.environ.get("BASS_TRACE"))
    res = run_bass_kernel_spmd(nc, in_maps, core_ids=list(range(NCORES)),
                               trace=trace)
    LAST_EXEC_NS = res.exec_time_ns
    out = np.concatenate(
        [np.asarray(res.results[c]["out"]).astype(np.float32)
         .reshape(128, 16, E).transpose(1, 0, 2)
         .reshape(BT_PER_CORE, NTOK, E) for c in range(NCORES)], axis=0)
    return out.reshape(B, T, NTOK, E)



# revision 73
# speedup vs baseline: 1.0346x; 1.0100x over previous
"""AFNO layer Trainium2 kernel — data-parallel over the 16 (b,t) pairs, 2 per core.

Pipeline per (b,t), per core (all matmuls bf16, accumulate f32):
  LN1 (token-major, batched stats) -> fwd DFT to 288 kept modes (matmul vs
  precomputed cos/sin, output channel-major) -> block-diag complex mixing
  (packed 128x128 matmuls, gelu / softshrink epilogues) -> PE transpose ->
  inverse DFT (matmul, token-major) -> +h +x residual -> LN2 -> PE transpose
  -> MLP (768->3072 gelu ->768) -> +res2.

Host-side folds: ln1_g into w1 (per-block diag), ln1_b vanishes in kept modes
(kx=5..28 excludes 0), ln2_g/ln2_b into mw1/mb1. All constants are
host-transposed into single contiguous SBUF images (one DMA each, issued on
gpsimd so the sync queue serves the activations first).
"""

import numpy as np
import ml_dtypes

B, T, NX, NY, E, BS = 2, 8, 32, 32, 768, 64
NB = E // BS
YM = NY // 2 + 1
KM = 12
LAM = 0.01
MODES = 24 * KM          # 288 kept modes
NTOK = NX * NY           # 1024 tokens per (b,t)
BT_PER_CORE = 2
NCORES = 8
TOK_CORE = BT_PER_CORE * NTOK   # 2048
H4 = 4 * E               # 3072
EPS = 1e-5

_BF16 = ml_dtypes.bfloat16
_FP8 = ml_dtypes.float8_e4m3
S1 = 64.0   # host scale on mw1 (fp8 needs values ~1; folded out in gelu scale)
S2 = 64.0   # host scale on mw2 (folded out in the residual-add epilogue)

_CACHE = {}


def _install_trace_shim():
    """Best-effort: register the axon NTFF profiling hook so BASS_TRACE=1 works."""
    try:
        import types, sys
        if 'antenv.axon_hooks' in sys.modules:
            return
        import antenv  # noqa
        from trn_agent_boot.trn_boot import _ntff_profile_via_ctypes
        mod = types.ModuleType('antenv.axon_hooks')
        hook = _ntff_profile_via_ctypes('/opt/axon/libaxon_pjrt.so')
        mod.get_axon_ntff_profile_hook = lambda: hook
        mod.set_axon_ntff_profile_hook = lambda h: None
        sys.modules['antenv.axon_hooks'] = mod
        from concourse import bass_utils
        bass_utils.upload_artifacts = lambda tmpdir: tmpdir
    except Exception:
        pass


def _dft_matrices():
    """ArT (1024,288), AiT, BrT (288,1024), BiT as float32 (analytic, ortho norm)."""
    xx, yy = np.meshgrid(np.arange(NX), np.arange(NY), indexing='ij')
    sx = xx.ravel().astype(np.float64)
    sy = yy.ravel().astype(np.float64)
    kxs = np.arange(YM - KM, YM + KM, dtype=np.float64)   # 5..28
    kys = np.arange(KM, dtype=np.float64)                  # 0..11
    KX, KY = np.meshgrid(kxs, kys, indexing='ij')
    mkx = KX.ravel()
    mky = KY.ravel()
    ph = 2 * np.pi * (np.outer(sx, mkx) + np.outer(sy, mky)) / 32.0  # (1024,288)
    art = np.cos(ph) / 32.0
    ait = -np.sin(ph) / 32.0
    wk = np.where(mky == 0, 1.0, 2.0)
    brt = ((np.cos(ph) / 32.0) * wk).T.copy()   # (288,1024)
    bit = ((-np.sin(ph) / 32.0) * wk).T.copy()
    return (art.astype(np.float32), ait.astype(np.float32),
            brt.astype(np.float32), bit.astype(np.float32))


def _pack_blockdiag(w):
    """w: (NB,64,64) -> (6,128,128) pairs of blocks on the diagonal."""
    out = np.zeros((NB // 2, 2 * BS, 2 * BS), np.float32)
    for j in range(NB // 2):
        out[j, :BS, :BS] = w[2 * j]
        out[j, BS:, BS:] = w[2 * j + 1]
    return out


def _sb_image(a, p):
    """(n*p, f) -> (p, n*f): partition-major SBUF image for one big DMA."""
    n = a.shape[0] // p
    return np.ascontiguousarray(
        a.reshape(n, p, a.shape[1]).transpose(1, 0, 2).reshape(p, n * a.shape[1]))


def _build_program(ln1_trivial, mb2_zero, stage=5):
    import concourse.bass as bass
    import concourse.bacc as bacc
    import concourse.mybir as mybir
    from concourse import tile
    from concourse.tile import add_dep_helper

    f32 = mybir.dt.float32
    bf16 = mybir.dt.bfloat16
    fp8 = mybir.dt.float8e4
    DR = mybir.MatmulPerfMode.DoubleRow
    AF = mybir.ActivationFunctionType
    ALU = mybir.AluOpType
    AX = mybir.AxisListType

    nc = bacc.Bacc("TRN2", target_bir_lowering=False, debug=False)

    dp = nc.declare_dram_parameter
    # x/out are host-transposed to partition-major [128, 16*E] so each DMA
    # descriptor covers a long contiguous per-partition row
    x_d = dp("x", [128, 16 * E], bf16, isOutput=False)
    art_d = dp("art", [128, 8 * MODES], fp8, isOutput=False)
    ait_d = dp("ait", [128, 8 * MODES], fp8, isOutput=False)
    bcp_d = dp("bcp", [128, 4 * NTOK], fp8, isOutput=False)
    brem_d = dp("brem", [128, 2 * NTOK], fp8, isOutput=False)
    wmix_d = dp("wmix", [128, 6 * 128 * 6], bf16, isOutput=False)  # 6 packed mats
    bmix_d = dp("bmix", [128, 36], f32, isOutput=False)  # b1r b1i b2rm b2rn b2im b2in
    mw1_d = dp("mw1f", [128, 6 * H4], fp8, isOutput=False)
    mb1_d = dp("mb1f", [128, 24], f32, isOutput=False)
    mw2_d = dp("mw2", [128, 24 * E], fp8, isOutput=False)
    ident_d = dp("ident", [128, 128], bf16, isOutput=False)
    if not ln1_trivial:
        g1r_d = dp("g1rep", [128, E], f32, isOutput=False)
        b1lr_d = dp("b1rep", [128, E], f32, isOutput=False)
    if not mb2_zero:
        mb2r_d = dp("mb2rep", [128, E], f32, isOutput=False)
    out_d = dp("out", [128, 16 * E], bf16, isOutput=True)

    with tile.TileContext(nc) as tc:
        with (
            tc.tile_pool(name="pc", bufs=1) as pc,
            tc.tile_pool(name="p4", bufs=3) as p4,
            tc.tile_pool(name="p2", bufs=2) as p2,
            tc.tile_pool(name="p8", bufs=8) as p8,
            tc.tile_pool(name="pfq", bufs=28) as pfq,
            tc.tile_pool(name="po2", bufs=3) as po2,
            tc.tile_pool(name="phid", bufs=24) as phid,
            tc.tile_pool(name="px2", bufs=6) as px2,
            tc.tile_pool(name="pst", bufs=4) as pst,
            tc.tile_pool(name="psum", bufs=2, space="PSUM") as pp,
        ):
            # ---- constants: DFT matrices + ident early (needed by phase B);
            # everything else deferred past the startup DMA burst ----
            art_all = pc.tile([128, 8 * MODES], fp8, tag="art")
            nc.gpsimd.dma_start(art_all[:], art_d[:])
            ait_all = pc.tile([128, 8 * MODES], fp8, tag="ait")
            nc.gpsimd.dma_start(ait_all[:], ait_d[:])
            ident_t = pc.tile([128, 128], bf16, tag="ident")
            nc.gpsimd.dma_start(ident_t[:], ident_d[:])
            wmix_all = pc.tile([128, 6 * 128 * 6], bf16, tag="wmix")
            bmix_all = pc.tile([128, 36], f32, tag="bmix")
            bcp_all = pc.tile([128, 4 * NTOK], fp8, tag="bcp")
            brem_all = pc.tile([128, 2 * NTOK], fp8, tag="brem")
            mw1_all = pc.tile([128, 6 * H4], fp8, tag="mw1")
            mw2_all = pc.tile([128, 24 * E], fp8, tag="mw2")
            mb1_all = pc.tile([128, 24], f32, tag="mb1")
            eps_t = pc.tile([128, 1], f32, tag="epsc")
            nc.vector.memset(eps_t[:], EPS)
            # warm the scalar-engine activation tables (Sqrt/Gelu/Relu) during
            # the input DMA so the first LN1 Sqrt isn't gated by a table load
            warm = pc.tile([128, 1], f32, tag="warm")
            nc.scalar.activation(warm[:], eps_t[:], AF.Sqrt, bias=eps_t[:])
            nc.scalar.activation(warm[:], eps_t[:], AF.Gelu, bias=eps_t[:])
            nc.scalar.activation(warm[:], eps_t[:], AF.Relu, bias=eps_t[:])
            nc.scalar.activation(warm[:], eps_t[:], AF.Identity, bias=eps_t[:])
            if not ln1_trivial:
                g1rep_t = pc.tile([128, E], f32, tag="g1rep")
                nc.gpsimd.dma_start(g1rep_t[:], g1r_d[:])
                b1rep_t = pc.tile([128, E], f32, tag="b1rep")
                nc.gpsimd.dma_start(b1rep_t[:], b1lr_d[:])
            if not mb2_zero:
                mb2rep_t = pc.tile([128, E], f32, tag="mb2rep")
                nc.gpsimd.dma_start(mb2rep_t[:], mb2r_d[:])

            artv = art_all[:].rearrange("p (q k m) -> p q k m", q=4, k=2)
            aitv = ait_all[:].rearrange("p (q k m) -> p q k m", q=4, k=2)
            bcpv = bcp_all[:].rearrange("p (c k t) -> p c k t", c=2, k=2)
            bremv = brem_all[:].rearrange("p (k t) -> p k t", k=2)
            # wmix order: w1r w1i w1in w2r w2i w2in, each (128, 6*128)
            def wm(idx, j):
                o = idx * 6 * 128 + j * 128
                return wmix_all[:, o:o + 128]
            def bm(idx, j):
                return bmix_all[:, idx * 6 + j: idx * 6 + j + 1]
            mw1v = mw1_all[:].rearrange("p (q k f) -> p q k f", q=3, k=2)
            mw2v = mw2_all[:].rearrange("p (q k e) -> p q k e", q=12, k=2)
            def mw1_c(q, fj):
                return mw1v[:, q, :, fj * 128:(fj + 1) * 128]
            def mw2_c(q, n):
                return mw2v[:, q, :, n * 384:(n + 1) * 384]
            def mb1_c(fj):
                return mb1_all[:, fj:fj + 1]

            def layernorm(src_aps, dst_pool, dst_tag, bt=0, nameprefix=None,
                          pair_fp8=False):
                """LN over 8 (128,E) APs via bn_stats -> normalized tiles.
                pair_fp8: write into 4 [128,2,E] fp8 pair tiles (DoubleRow layout).
                Returns (outs, rstds, nmrs) so hx emission can be deferred."""
                outs = []
                rstds = []
                nmrs = []
                for i in range(8):
                    xt = src_aps[i]
                    xr = xt.rearrange("p (n f) -> p n f", f=256)
                    stats = pst.tile([128, 3, 6], f32, tag="bst")
                    for s3 in range(3):
                        nc.vector.bn_stats(stats[:, s3, :], xr[:, s3, :])
                    mv = pst.tile([128, 2], f32, tag="mv")
                    nc.vector.bn_aggr(mv[:], stats[:])
                    std = pst.tile([128, 1], f32, tag="std")
                    nc.scalar.activation(std[:], mv[:, 1:2], AF.Sqrt, bias=eps_t[:])
                    rstd = pst.tile([128, 1], f32, tag="rstd", bufs=10)
                    nc.vector.reciprocal(rstd[:], std[:])
                    nmr = pst.tile([128, 1], f32, tag="nmr", bufs=10)
                    nc.vector.scalar_tensor_tensor(nmr[:], mv[:, 0:1], -1.0, rstd[:],
                                                   op0=ALU.mult, op1=ALU.mult)
                    rstds.append(rstd)
                    nmrs.append(nmr)
                    if pair_fp8:
                        if i % 2 == 0:
                            hpt = dst_pool.tile([128, 2, E], fp8, tag="hb", bufs=16,
                                                name=f"{nameprefix or dst_tag}_{bt}_{i // 2}")
                            outs.append(hpt)
                        dst_ap = outs[i // 2][:, i % 2, :]
                    else:
                        hb = dst_pool.tile([128, E], bf16, tag="hb", bufs=16,
                                           name=f"{nameprefix or dst_tag}_{bt}_{i}")
                        outs.append(hb)
                        dst_ap = hb[:]
                    if i % 3 == 2:
                        nc.scalar.activation(dst_ap, xt, AF.Identity,
                                             bias=nmr[:], scale=rstd[:])
                    else:
                        weng = nc.gpsimd if i % 3 == 0 else nc.vector
                        weng.tensor_scalar(dst_ap, xt, rstd[:], nmr[:],
                                           op0=ALU.mult, op1=ALU.add)
                return outs, rstds, nmrs

            def emit_hx(bt):
                """hx = h + x = x*(rstd+1) + nmr, off the critical path on gpsimd
                (only needed by phase D)."""
                xts = st[bt]['xts']
                rstds = st[bt]['rstds']
                nmrs = st[bt]['nmrs']
                hxs = []
                for i in range(8):
                    hxt = p8.tile([128, E], bf16, tag="hx", bufs=16,
                                  name=f"hx_{bt}_{i}")
                    if ln1_trivial:
                        r1p = pst.tile([128, 1], f32, tag="r1p", bufs=10)
                        nc.gpsimd.tensor_scalar_add(r1p[:], rstds[i][:], 1.0)
                        nc.gpsimd.tensor_scalar(hxt[:], xts[i], r1p[:], nmrs[i][:],
                                                op0=ALU.mult, op1=ALU.add)
                    else:
                        tmp = p2.tile([128, E], f32, tag="lngtmp")
                        nc.gpsimd.tensor_scalar(tmp[:], xts[i], rstds[i][:],
                                                nmrs[i][:], op0=ALU.mult, op1=ALU.add)
                        nc.gpsimd.tensor_tensor(tmp[:], tmp[:], g1rep_t[:],
                                                op=ALU.mult)
                        nc.gpsimd.tensor_tensor(tmp[:], tmp[:], b1rep_t[:],
                                                op=ALU.add)
                        nc.gpsimd.tensor_tensor(hxt[:], tmp[:], xts[i],
                                                op=ALU.add)
                    hxs.append(hxt)
                st[bt]['hx'] = hxs

            # ---- phase-interleaved pipeline over the two (b,t) shards: issue
            # order A0 A1 B0 B1 C0 C1 D0 E0 D1 E1 F00 F10 F01 F11 so one
            # shard's matmuls cover the other's LN/epilogue latency bubbles
            st = [dict() for _ in range(BT_PER_CORE)]

            def phase_A(bt):
                # x arrives host-transposed: chunk k of 128 tokens lives at
                # x_d[:, (bt*8+k)*E:(bt*8+k+1)*E]; 8 outstanding DMAs so the
                # descriptor chains fan out across DMA engines
                xts = []
                for i in range(8):
                    t = p8.tile([128, E], bf16, tag="xin", bufs=16,
                                name=f"x_{bt}_{i}")
                    c0 = (bt * 8 + i) * E
                    eng = nc.sync if i % 2 == 0 else nc.scalar
                    eng.dma_start(t[:], x_d[:, c0:c0 + E])
                    xts.append(t[:])
                hbf, rstds, nmrs = layernorm(xts, p8, "hb", bt=bt, pair_fp8=True)
                st[bt]['xts'] = xts
                st[bt]['hbf'] = hbf
                st[bt]['rstds'] = rstds
                st[bt]['nmrs'] = nmrs

            def phase_B(bt):
                # fwd DFT: FR/FI channel-major (e-chunk 128, 288), fp8 DoubleRow
                # over token pairs; psum holds 32*fr (art stored unscaled cos,
                # the /32 folded into w1 on host)
                hbf = st[bt]['hbf']
                frb = []
                fib = []
                last = None
                for j in range(6):
                    pfr = pp.tile([128, MODES], f32, tag="mmA", bufs=2)
                    for q in range(4):
                        nc.tensor.matmul(pfr[:], hbf[q][:, :, j * 128:(j + 1) * 128],
                                         artv[:, q, :, :], start=(q == 0), stop=(q == 3),
                                         perf_mode=DR)
                    fr = pfq.tile([128, MODES], bf16, tag="fq", name=f"fr{bt}_{j}")
                    nc.scalar.activation(fr[:], pfr[:], AF.Copy)
                    frb.append(fr)
                    pfi = pp.tile([128, MODES], f32, tag="mmA", bufs=2)
                    for q in range(4):
                        nc.tensor.matmul(pfi[:], hbf[q][:, :, j * 128:(j + 1) * 128],
                                         aitv[:, q, :, :], start=(q == 0), stop=(q == 3),
                                         perf_mode=DR)
                    fi = pfq.tile([128, MODES], bf16, tag="fq", name=f"fi{bt}_{j}")
                    last = nc.scalar.activation(fi[:], pfi[:], AF.Copy)
                    fib.append(fi)
                st[bt]['frb'] = frb
                st[bt]['fib'] = fib
                return last

            def phase_C(bt):
                # mixing layer 1 (complex, gelu), layer 2 (+softshrink), then
                # transposes — three software-pipelined sub-loops so the PE
                # never waits on a same-j scalar epilogue. Shrunk output goes
                # mode-major for the DoubleRow iDFT: two [128,2,E] fp8 pair
                # tiles (slot0=real slot1=imag) + a zero-padded [128,2,E] fp8
                # remainder (modes 256:288 of r in parts 0:32, i in 32:64)
                frb = st[bt]['frb']
                fib = st[bt]['fib']
                o2p = [po2.tile([128, 2, E], fp8, tag="o2p", bufs=4,
                                name=f"o2p{bt}_{c}") for c in range(2)]
                o2rem = po2.tile([128, 2, E], fp8, tag="o2rem", bufs=2,
                                 name=f"o2rem{bt}")
                nc.gpsimd.memset(o2rem[:], 0.0)
                o1rs, o1is, srs, sis = [], [], [], []
                for j in range(6):
                    p1r = pp.tile([128, MODES], f32, tag="mmA", bufs=2)
                    nc.tensor.matmul(p1r[:], wm(0, j), frb[j][:], start=True, stop=False)
                    nc.tensor.matmul(p1r[:], wm(2, j), fib[j][:], start=False, stop=True)
                    o1r = pfq.tile([128, MODES], bf16, tag="fq", name=f"o1r{bt}_{j}")
                    nc.scalar.activation(o1r[:], p1r[:], AF.Gelu, bias=bm(0, j))
                    o1rs.append(o1r)
                    p1i = pp.tile([128, MODES], f32, tag="mmA", bufs=2)
                    nc.tensor.matmul(p1i[:], wm(1, j), frb[j][:], start=True, stop=False)
                    nc.tensor.matmul(p1i[:], wm(0, j), fib[j][:], start=False, stop=True)
                    o1i = pfq.tile([128, MODES], bf16, tag="fq", name=f"o1i{bt}_{j}")
                    nc.scalar.activation(o1i[:], p1i[:], AF.Gelu, bias=bm(1, j))
                    o1is.append(o1i)
                for j in range(6):
                    o1r, o1i = o1rs[j], o1is[j]
                    p2r = pp.tile([128, MODES], f32, tag="mmA", bufs=2)
                    nc.tensor.matmul(p2r[:], wm(3, j), o1r[:], start=True, stop=False)
                    nc.tensor.matmul(p2r[:], wm(5, j), o1i[:], start=False, stop=True)
                    t1 = p2.tile([128, MODES], bf16, tag="t1")
                    t2 = p2.tile([128, MODES], bf16, tag="t2")
                    nc.scalar.activation(t1[:], p2r[:], AF.Relu, bias=bm(2, j), scale=32.0)
                    nc.scalar.activation(t2[:], p2r[:], AF.Relu, bias=bm(3, j), scale=-32.0)
                    sr = pfq.tile([128, MODES], bf16, tag="fq", name=f"shr{bt}_{j}")
                    nc.gpsimd.tensor_sub(sr[:], t1[:], t2[:])
                    srs.append(sr)
                    p2i = pp.tile([128, MODES], f32, tag="mmA", bufs=2)
                    nc.tensor.matmul(p2i[:], wm(4, j), o1r[:], start=True, stop=False)
                    nc.tensor.matmul(p2i[:], wm(3, j), o1i[:], start=False, stop=True)
                    t3 = p2.tile([128, MODES], bf16, tag="t1")
                    t4 = p2.tile([128, MODES], bf16, tag="t2")
                    nc.scalar.activation(t3[:], p2i[:], AF.Relu, bias=bm(4, j), scale=32.0)
                    nc.scalar.activation(t4[:], p2i[:], AF.Relu, bias=bm(5, j), scale=-32.0)
                    si = pfq.tile([128, MODES], bf16, tag="fq", name=f"shi{bt}_{j}")
                    nc.gpsimd.tensor_sub(si[:], t3[:], t4[:])
                    sis.append(si)
                for j in range(6):
                    sr, si = srs[j], sis[j]
                    ceng = nc.vector
                    for c in range(2):
                        ptr = pp.tile([128, 128], bf16, tag="tpm")
                        nc.tensor.transpose(ptr[:], sr[:, c * 128:(c + 1) * 128], ident_t[:])
                        ceng.tensor_copy(o2p[c][:, 0, j * 128:(j + 1) * 128], ptr[:])
                        pti = pp.tile([128, 128], bf16, tag="tpm")
                        nc.tensor.transpose(pti[:], si[:, c * 128:(c + 1) * 128], ident_t[:])
                        ceng.tensor_copy(o2p[c][:, 1, j * 128:(j + 1) * 128], pti[:])
                    ptr = pp.tile([128, 128], bf16, tag="tpm")
                    nc.tensor.transpose(ptr[0:32, :], sr[:, 256:288], ident_t[:])
                    ceng.tensor_copy(o2rem[0:32, 0, j * 128:(j + 1) * 128], ptr[0:32, :])
                    pti = pp.tile([128, 128], bf16, tag="tpm")
                    nc.tensor.transpose(pti[0:32, :], si[:, 256:288], ident_t[:])
                    ceng.tensor_copy(o2rem[32:64, 0, j * 128:(j + 1) * 128], pti[0:32, :])
                st[bt]['o2p'] = o2p
                st[bt]['o2rem'] = o2rem

            def phase_D(bt):
                # inverse DFT + residual, in place: out1 = hx += spat
                # psum = 1024*spat (32x in bcp/brem, 32x in the shrunk modes)
                o2p = st[bt]['o2p']
                o2rem = st[bt]['o2rem']
                hx = st[bt]['hx']
                for p in range(8):
                    for n in range(2):
                        ps = pp.tile([128, 384], f32, tag="big", bufs=4)
                        for c in range(2):
                            nc.tensor.matmul(ps[:], bcpv[:, c, :, p * 128:(p + 1) * 128],
                                             o2p[c][:, :, n * 384:(n + 1) * 384],
                                             start=(c == 0), stop=False, perf_mode=DR)
                        nc.tensor.matmul(ps[:], bremv[:, :, p * 128:(p + 1) * 128],
                                         o2rem[:, :, n * 384:(n + 1) * 384],
                                         start=False, stop=True, perf_mode=DR)
                        if n == 0:
                            nc.vector.scalar_tensor_tensor(
                                hx[p][:, 0:384], ps[:], 1.0 / 1024.0,
                                hx[p][:, 0:384], op0=ALU.mult, op1=ALU.add)
                        else:
                            # keep vector free for LN2 stats: scalar drains the
                            # psum, gpsimd folds it into hx
                            sp = p2.tile([128, 384], bf16, tag="dsp")
                            nc.scalar.activation(sp[:], ps[:], AF.Copy,
                                                 scale=1.0 / 1024.0)
                            nc.gpsimd.tensor_tensor(
                                hx[p][:, 384:768], sp[:], hx[p][:, 384:768],
                                op=ALU.add)
                st[bt]['out1'] = hx

            def phase_E(bt):
                # LN2 -> h2 (normalized token-major bf16; affine folded into mw1/mb1)
                h2bf, _, _ = layernorm([t[:] for t in st[bt]['out1']], p8, "h2",
                                       bt=bt, nameprefix="h2")
                st[bt]['h2'] = h2bf

            def phase_F(bt, h):
                # MLP half: transpose h2 -> fp8 channel-major pairs, fp8 DoubleRow
                # 768->3072 gelu ->768, + res2, one 4-chunk DMA out
                h2bf = st[bt]['h2']
                out1 = st[bt]['out1']
                x2h = [px2.tile([128, 2, 512], fp8, tag="x2q", bufs=6,
                                name=f"x2h{bt}_{h}_{q}") for q in range(3)]
                for tcn in range(4):
                    p = h * 4 + tcn
                    for j in range(6):
                        pt = pp.tile([128, 128], bf16, tag="tpm")
                        nc.tensor.transpose(pt[:], h2bf[p][:, j * 128:(j + 1) * 128],
                                            ident_t[:])
                        nc.vector.tensor_copy(
                            x2h[j // 2][:, j % 2, tcn * 128:(tcn + 1) * 128], pt[:])
                hid = [phid.tile([128, 2, 512], fp8, tag="hid", bufs=24,
                                 name=f"hid{bt}_{h}_{qq}") for qq in range(12)]
                for fj in range(24):
                    ph = pp.tile([128, 512], f32, tag="big", bufs=4)
                    for q in range(3):
                        nc.tensor.matmul(ph[:], mw1_c(q, fj), x2h[q][:],
                                         start=(q == 0), stop=(q == 2),
                                         perf_mode=DR)
                    nc.scalar.activation(hid[fj // 2][:, fj % 2, :], ph[:],
                                         AF.Gelu, bias=mb1_c(fj), scale=1.0 / S1)
                for tcn in range(4):
                    p = h * 4 + tcn
                    ost = p8.tile([128, E], bf16, tag="xin", bufs=16,
                                  name=f"ost{bt}_{h}_{tcn}")
                    for n in range(2):
                        po = pp.tile([128, 384], f32, tag="big", bufs=4)
                        for qq in range(12):
                            nc.tensor.matmul(po[:],
                                             hid[qq][:, :, tcn * 128:(tcn + 1) * 128],
                                             mw2_c(qq, n),
                                             start=(qq == 0), stop=(qq == 11),
                                             perf_mode=DR)
                        nc.vector.scalar_tensor_tensor(
                            ost[:, n * 384:(n + 1) * 384], po[:], 1.0 / S2,
                            out1[p][:, n * 384:(n + 1) * 384],
                            op0=ALU.mult, op1=ALU.add)
                    if not mb2_zero:
                        nc.vector.tensor_add(ost[:], ost[:], mb2rep_t[:])
                    c0 = (bt * 8 + p) * E
                    oeng = (nc.sync, nc.scalar, nc.gpsimd)[p % 3]
                    oeng2 = (nc.scalar, nc.gpsimd, nc.sync)[p % 3]
                    oeng.dma_start(out_d[:, c0:c0 + E // 2], ost[:, 0:E // 2])
                    oeng2.dma_start(out_d[:, c0 + E // 2:c0 + E], ost[:, E // 2:E])

            phase_A(0)
            phase_A(1)
            fi_copy = phase_B(0)
            # deferred weight loads: don't let these race the startup burst
            # (x tiles + DFT matrices) on the HBM wire; wmix/bmix first (needed
            # by phase C right after B1)
            for dd_d, dd_t in ((wmix_d, wmix_all), (bmix_d, bmix_all),
                               (bcp_d, bcp_all), (brem_d, brem_all),
                               (mw1_d, mw1_all), (mw2_d, mw2_all),
                               (mb1_d, mb1_all)):
                dd = nc.gpsimd.dma_start(dd_t[:], dd_d[:])
                add_dep_helper(dd.ins, fi_copy.ins,
                               reason="defer bulk weight DMA past fwd DFT")
            phase_B(1)
            emit_hx(0)
            emit_hx(1)
            phase_C(0)
            phase_C(1)
            phase_D(0)
            phase_E(0)
            phase_D(1)
            phase_E(1)
            phase_F(0, 0)
            phase_F(1, 0)
            phase_F(0, 1)
            phase_F(1, 1)

    nc.compile()
    return nc


LAST_EXEC_NS = None


def make_consts(w1, b1, w2, b2, ln1_g, ln1_b, ln2_g, ln2_b,
                mw1, mb1, mw2, mb2, ln1_trivial, mb2_zero):
    art, ait, brt, bit = _dft_matrices()

    # fold ln1_g into w1 (left-diag per block over the i axis)
    g_blocks = ln1_g.reshape(NB, BS)
    W1R = _pack_blockdiag(w1[0] * g_blocks[:, :, None])
    W1I = _pack_blockdiag(w1[1] * g_blocks[:, :, None])
    W2R = _pack_blockdiag(w2[0])
    W2I = _pack_blockdiag(w2[1])

    b1r = b1[0].reshape(E)
    b1i = b1[1].reshape(E)
    b2r = b2[0].reshape(E)
    b2i = b2[1].reshape(E)

    mw1f = mw1 * ln2_g[:, None]
    mb1f = (mb1 + ln2_b @ mw1).reshape(H4)

    def bf(a):
        return np.ascontiguousarray(a.astype(_BF16))

    def fp8_pairs(a, scale):
        """(2q*128, F) -> (128, q*2*F) k-pair-interleaved fp8 image for DoubleRow."""
        nq = a.shape[0] // 256
        img = (a * scale).reshape(nq, 2, 128, a.shape[1]) \
            .transpose(2, 0, 1, 3).reshape(128, 2 * nq * a.shape[1])
        return np.ascontiguousarray(img.astype(_FP8))

    # wmix image: (128, 6 mats * 6 blocks * 128), order w1r w1i w1in w2r w2i w2in
    # w1 carries the 1/32 that was removed from the fp8 DFT matrices
    mats = [W1R / 32.0, W1I / 32.0, -W1I / 32.0, W2R, W2I, -W2I]
    wmix = np.concatenate(
        [m.transpose(1, 0, 2).reshape(128, 6 * 128) for m in mats], axis=1)
    # bmix image: (128, 36): 6 vectors x 6 chunks; shrink biases carry the
    # 32x fp8-friendly scale on the shrunk modes (undone by 1/1024 after iDFT)
    bvecs = [b1r, b1i, 32.0 * (b2r - LAM), 32.0 * (-b2r - LAM),
             32.0 * (b2i - LAM), 32.0 * (-b2i - LAM)]
    bmix = np.concatenate([v.reshape(6, 128).T for v in bvecs], axis=1)

    # iDFT pair image: [128, c(2), k(2), 1024] slot0=brt, slot1=bit (32x scale)
    brt32 = brt * 32.0
    bit32 = bit * 32.0
    bcp = np.zeros((128, 2, 2, NTOK), np.float32)
    for c in range(2):
        bcp[:, c, 0, :] = brt32[c * 128:(c + 1) * 128]
        bcp[:, c, 1, :] = bit32[c * 128:(c + 1) * 128]
    # remainder modes 256:288 as a zero-padded DoubleRow pair (r parts 0:32,
    # i parts 32:64, slot1 all zero)
    brem = np.zeros((128, 2, NTOK), np.float32)
    brem[0:32, 0, :] = brt32[256:288]
    brem[32:64, 0, :] = bit32[256:288]

    consts = {
        "art": fp8_pairs(art, 32.0), "ait": fp8_pairs(ait, 32.0),
        "bcp": np.ascontiguousarray(bcp.reshape(128, 4 * NTOK).astype(_FP8)),
        "brem": np.ascontiguousarray(brem.reshape(128, 2 * NTOK).astype(_FP8)),
        "wmix": bf(wmix), "bmix": np.ascontiguousarray(bmix, np.float32),
        "mw1f": fp8_pairs(mw1f, S1),
        "mb1f": np.ascontiguousarray(mb1f.reshape(24, 128).T, np.float32),
        "mw2": fp8_pairs(mw2, S2),
        "ident": bf(np.eye(128, dtype=np.float32)),
    }
    if not ln1_trivial:
        consts["g1rep"] = np.tile(ln1_g[None, :], (128, 1)).astype(np.float32)
        consts["b1rep"] = np.tile(ln1_b[None, :], (128, 1)).astype(np.float32)
    if not mb2_zero:
        consts["mb2rep"] = np.tile(mb2[None, :], (128, 1)).astype(np.float32)
    return consts


def kernel(input, w1, b1, w2, b2, ln1_g, ln1_b, ln2_g, ln2_b, mw1, mb1, mw2, mb2):
    global LAST_EXEC_NS
    _install_trace_shim()
    import os
    from concourse.bass_utils import run_bass_kernel_spmd

    input = np.asarray(input, np.float32)
    w1 = np.asarray(w1, np.float32)
    b1 = np.asarray(b1, np.float32)
    w2 = np.asarray(w2, np.float32)
    b2 = np.asarray(b2, np.float32)
    ln1_g = np.asarray(ln1_g, np.float32)
    ln1_b = np.asarray(ln1_b, np.float32)
    ln2_g = np.asarray(ln2_g, np.float32)
    ln2_b = np.asarray(ln2_b, np.float32)
    mw1 = np.asarray(mw1, np.float32)
    mb1 = np.asarray(mb1, np.float32)
    mw2 = np.asarray(mw2, np.float32)
    mb2 = np.asarray(mb2, np.float32)

    ln1_trivial = bool(np.all(ln1_g == 1.0) and np.all(ln1_b == 0.0))
    mb2_zero = bool(np.all(mb2 == 0.0))

    key = (ln1_trivial, mb2_zero)
    if key not in _CACHE:
        _CACHE[key] = _build_program(ln1_trivial, mb2_zero)
    nc = _CACHE[key]

    consts = make_consts(w1, b1, w2, b2, ln1_g, ln1_b, ln2_g, ln2_b,
                         mw1, mb1, mw2, mb2, ln1_trivial, mb2_zero)

    xs = input.reshape(B * T, NTOK, E)
    in_maps = []
    for c in range(NCORES):
        shard = xs[c * BT_PER_CORE:(c + 1) * BT_PER_CORE].reshape(TOK_CORE, E)
        # partition-major image: img[p, k*E:(k+1)*E] = x[k*128+p, :]
        img = np.ascontiguousarray(
            shard.reshape(16, 128, E).transpose(1, 0, 2).reshape(128, 16 * E)
            .astype(_BF16))
        m = {"x": img}
        m.update(consts)
        in_maps.append(m)

    trace = bool(os.environ.get("BASS_TRACE"))
    res = run_bass_kernel_spmd(nc, in_maps, core_ids=list(range(NCORES)),
                               trace=trace)
    LAST_EXEC_NS = res.exec_time_ns
    out = np.concatenate(
        [np.asarray(res.results[c]["out"]).astype(np.float32)
         .reshape(128, 16, E).transpose(1, 0, 2)
         .reshape(BT_PER_CORE, NTOK, E) for c in range(NCORES)], axis=0)
    return out.reshape(B, T, NTOK, E)



# revision 74
# speedup vs baseline: 1.0353x; 1.0007x over previous
"""AFNO layer Trainium2 kernel — data-parallel over the 16 (b,t) pairs, 2 per core.

Pipeline per (b,t), per core (all matmuls bf16, accumulate f32):
  LN1 (token-major, batched stats) -> fwd DFT to 288 kept modes (matmul vs
  precomputed cos/sin, output channel-major) -> block-diag complex mixing
  (packed 128x128 matmuls, gelu / softshrink epilogues) -> PE transpose ->
  inverse DFT (matmul, token-major) -> +h +x residual -> LN2 -> PE transpose
  -> MLP (768->3072 gelu ->768) -> +res2.

Host-side folds: ln1_g into w1 (per-block diag), ln1_b vanishes in kept modes
(kx=5..28 excludes 0), ln2_g/ln2_b into mw1/mb1. All constants are
host-transposed into single contiguous SBUF images (one DMA each, issued on
gpsimd so the sync queue serves the activations first).
"""

import numpy as np
import ml_dtypes

B, T, NX, NY, E, BS = 2, 8, 32, 32, 768, 64
NB = E // BS
YM = NY // 2 + 1
KM = 12
LAM = 0.01
MODES = 24 * KM          # 288 kept modes
NTOK = NX * NY           # 1024 tokens per (b,t)
BT_PER_CORE = 2
NCORES = 8
TOK_CORE = BT_PER_CORE * NTOK   # 2048
H4 = 4 * E               # 3072
EPS = 1e-5

_BF16 = ml_dtypes.bfloat16
_FP8 = ml_dtypes.float8_e4m3
S1 = 64.0   # host scale on mw1 (fp8 needs values ~1; folded out in gelu scale)
S2 = 64.0   # host scale on mw2 (folded out in the residual-add epilogue)

_CACHE = {}


def _install_trace_shim():
    """Best-effort: register the axon NTFF profiling hook so BASS_TRACE=1 works."""
    try:
        import types, sys
        if 'antenv.axon_hooks' in sys.modules:
            return
        import antenv  # noqa
        from trn_agent_boot.trn_boot import _ntff_profile_via_ctypes
        mod = types.ModuleType('antenv.axon_hooks')
        hook = _ntff_profile_via_ctypes('/opt/axon/libaxon_pjrt.so')
        mod.get_axon_ntff_profile_hook = lambda: hook
        mod.set_axon_ntff_profile_hook = lambda h: None
        sys.modules['antenv.axon_hooks'] = mod
        from concourse import bass_utils
        bass_utils.upload_artifacts = lambda tmpdir: tmpdir
    except Exception:
        pass


def _dft_matrices():
    """ArT (1024,288), AiT, BrT (288,1024), BiT as float32 (analytic, ortho norm)."""
    xx, yy = np.meshgrid(np.arange(NX), np.arange(NY), indexing='ij')
    sx = xx.ravel().astype(np.float64)
    sy = yy.ravel().astype(np.float64)
    kxs = np.arange(YM - KM, YM + KM, dtype=np.float64)   # 5..28
    kys = np.arange(KM, dtype=np.float64)                  # 0..11
    KX, KY = np.meshgrid(kxs, kys, indexing='ij')
    mkx = KX.ravel()
    mky = KY.ravel()
    ph = 2 * np.pi * (np.outer(sx, mkx) + np.outer(sy, mky)) / 32.0  # (1024,288)
    art = np.cos(ph) / 32.0
    ait = -np.sin(ph) / 32.0
    wk = np.where(mky == 0, 1.0, 2.0)
    brt = ((np.cos(ph) / 32.0) * wk).T.copy()   # (288,1024)
    bit = ((-np.sin(ph) / 32.0) * wk).T.copy()
    return (art.astype(np.float32), ait.astype(np.float32),
            brt.astype(np.float32), bit.astype(np.float32))


def _pack_blockdiag(w):
    """w: (NB,64,64) -> (6,128,128) pairs of blocks on the diagonal."""
    out = np.zeros((NB // 2, 2 * BS, 2 * BS), np.float32)
    for j in range(NB // 2):
        out[j, :BS, :BS] = w[2 * j]
        out[j, BS:, BS:] = w[2 * j + 1]
    return out


def _sb_image(a, p):
    """(n*p, f) -> (p, n*f): partition-major SBUF image for one big DMA."""
    n = a.shape[0] // p
    return np.ascontiguousarray(
        a.reshape(n, p, a.shape[1]).transpose(1, 0, 2).reshape(p, n * a.shape[1]))


def _build_program(ln1_trivial, mb2_zero, stage=5):
    import concourse.bass as bass
    import concourse.bacc as bacc
    import concourse.mybir as mybir
    from concourse import tile
    from concourse.tile import add_dep_helper

    f32 = mybir.dt.float32
    bf16 = mybir.dt.bfloat16
    fp8 = mybir.dt.float8e4
    DR = mybir.MatmulPerfMode.DoubleRow
    AF = mybir.ActivationFunctionType
    ALU = mybir.AluOpType
    AX = mybir.AxisListType

    nc = bacc.Bacc("TRN2", target_bir_lowering=False, debug=False)

    dp = nc.declare_dram_parameter
    # x/out are host-transposed to partition-major [128, 16*E] so each DMA
    # descriptor covers a long contiguous per-partition row
    x_d = dp("x", [128, 16 * E], bf16, isOutput=False)
    art_d = dp("art", [128, 8 * MODES], fp8, isOutput=False)
    ait_d = dp("ait", [128, 8 * MODES], fp8, isOutput=False)
    bcp_d = dp("bcp", [128, 4 * NTOK], fp8, isOutput=False)
    brem_d = dp("brem", [128, 2 * NTOK], fp8, isOutput=False)
    wmix_d = dp("wmix", [128, 6 * 128 * 6], bf16, isOutput=False)  # 6 packed mats
    bmix_d = dp("bmix", [128, 36], f32, isOutput=False)  # b1r b1i b2rm b2rn b2im b2in
    mw1_d = dp("mw1f", [128, 6 * H4], fp8, isOutput=False)
    mb1_d = dp("mb1f", [128, 24], f32, isOutput=False)
    mw2_d = dp("mw2", [128, 24 * E], fp8, isOutput=False)
    ident_d = dp("ident", [128, 128], bf16, isOutput=False)
    if not ln1_trivial:
        g1r_d = dp("g1rep", [128, E], f32, isOutput=False)
        b1lr_d = dp("b1rep", [128, E], f32, isOutput=False)
    if not mb2_zero:
        mb2r_d = dp("mb2rep", [128, E], f32, isOutput=False)
    out_d = dp("out", [128, 16 * E], bf16, isOutput=True)

    with tile.TileContext(nc) as tc:
        with (
            tc.tile_pool(name="pc", bufs=1) as pc,
            tc.tile_pool(name="p4", bufs=3) as p4,
            tc.tile_pool(name="p2", bufs=2) as p2,
            tc.tile_pool(name="p8", bufs=8) as p8,
            tc.tile_pool(name="pfq", bufs=28) as pfq,
            tc.tile_pool(name="po2", bufs=3) as po2,
            tc.tile_pool(name="phid", bufs=24) as phid,
            tc.tile_pool(name="px2", bufs=6) as px2,
            tc.tile_pool(name="pst", bufs=4) as pst,
            tc.tile_pool(name="psum", bufs=2, space="PSUM") as pp,
        ):
            # ---- constants: DFT matrices + ident early (needed by phase B);
            # everything else deferred past the startup DMA burst ----
            art_all = pc.tile([128, 8 * MODES], fp8, tag="art")
            nc.gpsimd.dma_start(art_all[:], art_d[:])
            ait_all = pc.tile([128, 8 * MODES], fp8, tag="ait")
            nc.gpsimd.dma_start(ait_all[:], ait_d[:])
            ident_t = pc.tile([128, 128], bf16, tag="ident")
            nc.gpsimd.dma_start(ident_t[:], ident_d[:])
            wmix_all = pc.tile([128, 6 * 128 * 6], bf16, tag="wmix")
            bmix_all = pc.tile([128, 36], f32, tag="bmix")
            bcp_all = pc.tile([128, 4 * NTOK], fp8, tag="bcp")
            brem_all = pc.tile([128, 2 * NTOK], fp8, tag="brem")
            mw1_all = pc.tile([128, 6 * H4], fp8, tag="mw1")
            mw2_all = pc.tile([128, 24 * E], fp8, tag="mw2")
            mb1_all = pc.tile([128, 24], f32, tag="mb1")
            eps_t = pc.tile([128, 1], f32, tag="epsc")
            nc.vector.memset(eps_t[:], EPS)
            # warm the scalar-engine activation tables (Sqrt/Gelu/Relu) during
            # the input DMA so the first LN1 Sqrt isn't gated by a table load
            warm = pc.tile([128, 1], f32, tag="warm")
            nc.scalar.activation(warm[:], eps_t[:], AF.Sqrt, bias=eps_t[:])
            nc.scalar.activation(warm[:], eps_t[:], AF.Gelu, bias=eps_t[:])
            nc.scalar.activation(warm[:], eps_t[:], AF.Relu, bias=eps_t[:])
            nc.scalar.activation(warm[:], eps_t[:], AF.Identity, bias=eps_t[:])
            if not ln1_trivial:
                g1rep_t = pc.tile([128, E], f32, tag="g1rep")
                nc.gpsimd.dma_start(g1rep_t[:], g1r_d[:])
                b1rep_t = pc.tile([128, E], f32, tag="b1rep")
                nc.gpsimd.dma_start(b1rep_t[:], b1lr_d[:])
            if not mb2_zero:
                mb2rep_t = pc.tile([128, E], f32, tag="mb2rep")
                nc.gpsimd.dma_start(mb2rep_t[:], mb2r_d[:])

            artv = art_all[:].rearrange("p (q k m) -> p q k m", q=4, k=2)
            aitv = ait_all[:].rearrange("p (q k m) -> p q k m", q=4, k=2)
            bcpv = bcp_all[:].rearrange("p (c k t) -> p c k t", c=2, k=2)
            bremv = brem_all[:].rearrange("p (k t) -> p k t", k=2)
            # wmix order: w1r w1i w1in w2r w2i w2in, each (128, 6*128)
            def wm(idx, j):
                o = idx * 6 * 128 + j * 128
                return wmix_all[:, o:o + 128]
            def bm(idx, j):
                return bmix_all[:, idx * 6 + j: idx * 6 + j + 1]
            mw1v = mw1_all[:].rearrange("p (q k f) -> p q k f", q=3, k=2)
            mw2v = mw2_all[:].rearrange("p (q k e) -> p q k e", q=12, k=2)
            def mw1_c(q, fj):
                return mw1v[:, q, :, fj * 128:(fj + 1) * 128]
            def mw2_c(q, n):
                return mw2v[:, q, :, n * 384:(n + 1) * 384]
            def mb1_c(fj):
                return mb1_all[:, fj:fj + 1]

            def layernorm(src_aps, dst_pool, dst_tag, bt=0, nameprefix=None,
                          pair_fp8=False):
                """LN over 8 (128,E) APs via bn_stats -> normalized tiles.
                pair_fp8: write into 4 [128,2,E] fp8 pair tiles (DoubleRow layout).
                Returns (outs, rstds, nmrs) so hx emission can be deferred."""
                outs = []
                rstds = []
                nmrs = []
                for i in range(8):
                    xt = src_aps[i]
                    xr = xt.rearrange("p (n f) -> p n f", f=256)
                    stats = pst.tile([128, 3, 6], f32, tag="bst")
                    for s3 in range(3):
                        nc.vector.bn_stats(stats[:, s3, :], xr[:, s3, :])
                    mv = pst.tile([128, 2], f32, tag="mv")
                    nc.vector.bn_aggr(mv[:], stats[:])
                    std = pst.tile([128, 1], f32, tag="std")
                    nc.scalar.activation(std[:], mv[:, 1:2], AF.Sqrt, bias=eps_t[:])
                    rstd = pst.tile([128, 1], f32, tag="rstd", bufs=10)
                    nc.vector.reciprocal(rstd[:], std[:])
                    nmr = pst.tile([128, 1], f32, tag="nmr", bufs=10)
                    nc.vector.scalar_tensor_tensor(nmr[:], mv[:, 0:1], -1.0, rstd[:],
                                                   op0=ALU.mult, op1=ALU.mult)
                    rstds.append(rstd)
                    nmrs.append(nmr)
                    if pair_fp8:
                        if i % 2 == 0:
                            hpt = dst_pool.tile([128, 2, E], fp8, tag="hb", bufs=16,
                                                name=f"{nameprefix or dst_tag}_{bt}_{i // 2}")
                            outs.append(hpt)
                        dst_ap = outs[i // 2][:, i % 2, :]
                    else:
                        hb = dst_pool.tile([128, E], bf16, tag="hb", bufs=16,
                                           name=f"{nameprefix or dst_tag}_{bt}_{i}")
                        outs.append(hb)
                        dst_ap = hb[:]
                    if i % 3 == 2:
                        nc.scalar.activation(dst_ap, xt, AF.Identity,
                                             bias=nmr[:], scale=rstd[:])
                    else:
                        weng = nc.gpsimd if i % 3 == 0 else nc.vector
                        weng.tensor_scalar(dst_ap, xt, rstd[:], nmr[:],
                                           op0=ALU.mult, op1=ALU.add)
                return outs, rstds, nmrs

            def emit_hx(bt):
                """hx = h + x = x*(rstd+1) + nmr, off the critical path on gpsimd
                (only needed by phase D)."""
                xts = st[bt]['xts']
                rstds = st[bt]['rstds']
                nmrs = st[bt]['nmrs']
                hxs = []
                for i in range(8):
                    hxt = p8.tile([128, E], bf16, tag="hx", bufs=16,
                                  name=f"hx_{bt}_{i}")
                    if ln1_trivial:
                        r1p = pst.tile([128, 1], f32, tag="r1p", bufs=10)
                        nc.gpsimd.tensor_scalar_add(r1p[:], rstds[i][:], 1.0)
                        nc.gpsimd.tensor_scalar(hxt[:], xts[i], r1p[:], nmrs[i][:],
                                                op0=ALU.mult, op1=ALU.add)
                    else:
                        tmp = p2.tile([128, E], f32, tag="lngtmp")
                        nc.gpsimd.tensor_scalar(tmp[:], xts[i], rstds[i][:],
                                                nmrs[i][:], op0=ALU.mult, op1=ALU.add)
                        nc.gpsimd.tensor_tensor(tmp[:], tmp[:], g1rep_t[:],
                                                op=ALU.mult)
                        nc.gpsimd.tensor_tensor(tmp[:], tmp[:], b1rep_t[:],
                                                op=ALU.add)
                        nc.gpsimd.tensor_tensor(hxt[:], tmp[:], xts[i],
                                                op=ALU.add)
                    hxs.append(hxt)
                st[bt]['hx'] = hxs

            # ---- phase-interleaved pipeline over the two (b,t) shards: issue
            # order A0 A1 B0 B1 C0 C1 D0 E0 D1 E1 F00 F10 F01 F11 so one
            # shard's matmuls cover the other's LN/epilogue latency bubbles
            st = [dict() for _ in range(BT_PER_CORE)]

            def phase_A(bt):
                # x arrives host-transposed: chunk k of 128 tokens lives at
                # x_d[:, (bt*8+k)*E:(bt*8+k+1)*E]; 8 outstanding DMAs so the
                # descriptor chains fan out across DMA engines
                xts = []
                for i in range(8):
                    t = p8.tile([128, E], bf16, tag="xin", bufs=16,
                                name=f"x_{bt}_{i}")
                    c0 = (bt * 8 + i) * E
                    eng = nc.sync if i % 2 == 0 else nc.scalar
                    eng.dma_start(t[:], x_d[:, c0:c0 + E])
                    xts.append(t[:])
                hbf, rstds, nmrs = layernorm(xts, p8, "hb", bt=bt, pair_fp8=True)
                st[bt]['xts'] = xts
                st[bt]['hbf'] = hbf
                st[bt]['rstds'] = rstds
                st[bt]['nmrs'] = nmrs

            def phase_B(bt):
                # fwd DFT: FR/FI channel-major (e-chunk 128, 288), fp8 DoubleRow
                # over token pairs; psum holds 32*fr (art stored unscaled cos,
                # the /32 folded into w1 on host)
                hbf = st[bt]['hbf']
                frb = []
                fib = []
                last = None
                for j in range(6):
                    pfr = pp.tile([128, MODES], f32, tag="mmA", bufs=2)
                    for q in range(4):
                        nc.tensor.matmul(pfr[:], hbf[q][:, :, j * 128:(j + 1) * 128],
                                         artv[:, q, :, :], start=(q == 0), stop=(q == 3),
                                         perf_mode=DR)
                    fr = pfq.tile([128, MODES], bf16, tag="fq", name=f"fr{bt}_{j}")
                    nc.scalar.activation(fr[:], pfr[:], AF.Copy)
                    frb.append(fr)
                    pfi = pp.tile([128, MODES], f32, tag="mmA", bufs=2)
                    for q in range(4):
                        nc.tensor.matmul(pfi[:], hbf[q][:, :, j * 128:(j + 1) * 128],
                                         aitv[:, q, :, :], start=(q == 0), stop=(q == 3),
                                         perf_mode=DR)
                    fi = pfq.tile([128, MODES], bf16, tag="fq", name=f"fi{bt}_{j}")
                    last = nc.scalar.activation(fi[:], pfi[:], AF.Copy)
                    fib.append(fi)
                st[bt]['frb'] = frb
                st[bt]['fib'] = fib
                return last

            def phase_C(bt):
                # mixing layer 1 (complex, gelu), layer 2 (+softshrink), then
                # transposes — three software-pipelined sub-loops so the PE
                # never waits on a same-j scalar epilogue. Shrunk output goes
                # mode-major for the DoubleRow iDFT: two [128,2,E] fp8 pair
                # tiles (slot0=real slot1=imag) + a zero-padded [128,2,E] fp8
                # remainder (modes 256:288 of r in parts 0:32, i in 32:64)
                frb = st[bt]['frb']
                fib = st[bt]['fib']
                o2p = [po2.tile([128, 2, E], fp8, tag="o2p", bufs=4,
                                name=f"o2p{bt}_{c}") for c in range(2)]
                o2rem = po2.tile([128, 2, E], fp8, tag="o2rem", bufs=2,
                                 name=f"o2rem{bt}")
                nc.gpsimd.memset(o2rem[:], 0.0)
                o1rs, o1is, srs, sis = [], [], [], []
                for j in range(6):
                    p1r = pp.tile([128, MODES], f32, tag="mmA", bufs=2)
                    nc.tensor.matmul(p1r[:], wm(0, j), frb[j][:], start=True, stop=False)
                    nc.tensor.matmul(p1r[:], wm(2, j), fib[j][:], start=False, stop=True)
                    o1r = pfq.tile([128, MODES], bf16, tag="fq", name=f"o1r{bt}_{j}")
                    nc.scalar.activation(o1r[:], p1r[:], AF.Gelu, bias=bm(0, j))
                    o1rs.append(o1r)
                    p1i = pp.tile([128, MODES], f32, tag="mmA", bufs=2)
                    nc.tensor.matmul(p1i[:], wm(1, j), frb[j][:], start=True, stop=False)
                    nc.tensor.matmul(p1i[:], wm(0, j), fib[j][:], start=False, stop=True)
                    o1i = pfq.tile([128, MODES], bf16, tag="fq", name=f"o1i{bt}_{j}")
                    nc.scalar.activation(o1i[:], p1i[:], AF.Gelu, bias=bm(1, j))
                    o1is.append(o1i)
                for j in range(6):
                    o1r, o1i = o1rs[j], o1is[j]
                    p2r = pp.tile([128, MODES], f32, tag="mmA", bufs=2)
                    nc.tensor.matmul(p2r[:], wm(3, j), o1r[:], start=True, stop=False)
                    nc.tensor.matmul(p2r[:], wm(5, j), o1i[:], start=False, stop=True)
                    t1 = p2.tile([128, MODES], bf16, tag="t1")
                    t2 = p2.tile([128, MODES], bf16, tag="t2")
                    nc.scalar.activation(t1[:], p2r[:], AF.Relu, bias=bm(2, j), scale=32.0)
                    nc.scalar.activation(t2[:], p2r[:], AF.Relu, bias=bm(3, j), scale=-32.0)
                    sr = pfq.tile([128, MODES], bf16, tag="fq", name=f"shr{bt}_{j}")
                    nc.gpsimd.tensor_sub(sr[:], t1[:], t2[:])
                    srs.append(sr)
                    p2i = pp.tile([128, MODES], f32, tag="mmA", bufs=2)
                    nc.tensor.matmul(p2i[:], wm(4, j), o1r[:], start=True, stop=False)
                    nc.tensor.matmul(p2i[:], wm(3, j), o1i[:], start=False, stop=True)
                    t3 = p2.tile([128, MODES], bf16, tag="t1")
                    t4 = p2.tile([128, MODES], bf16, tag="t2")
                    nc.scalar.activation(t3[:], p2i[:], AF.Relu, bias=bm(4, j), scale=32.0)
                    nc.scalar.activation(t4[:], p2i[:], AF.Relu, bias=bm(5, j), scale=-32.0)
                    si = pfq.tile([128, MODES], bf16, tag="fq", name=f"shi{bt}_{j}")
                    nc.gpsimd.tensor_sub(si[:], t3[:], t4[:])
                    sis.append(si)
                for j in range(6):
                    sr, si = srs[j], sis[j]
                    ceng = nc.vector
                    for c in range(2):
                        ptr = pp.tile([128, 128], bf16, tag="tpm")
                        nc.tensor.transpose(ptr[:], sr[:, c * 128:(c + 1) * 128], ident_t[:])
                        ceng.tensor_copy(o2p[c][:, 0, j * 128:(j + 1) * 128], ptr[:])
                        pti = pp.tile([128, 128], bf16, tag="tpm")
                        nc.tensor.transpose(pti[:], si[:, c * 128:(c + 1) * 128], ident_t[:])
                        ceng.tensor_copy(o2p[c][:, 1, j * 128:(j + 1) * 128], pti[:])
                    ptr = pp.tile([128, 128], bf16, tag="tpm")
                    nc.tensor.transpose(ptr[0:32, :], sr[:, 256:288], ident_t[:])
                    ceng.tensor_copy(o2rem[0:32, 0, j * 128:(j + 1) * 128], ptr[0:32, :])
                    pti = pp.tile([128, 128], bf16, tag="tpm")
                    nc.tensor.transpose(pti[0:32, :], si[:, 256:288], ident_t[:])
                    ceng.tensor_copy(o2rem[32:64, 0, j * 128:(j + 1) * 128], pti[0:32, :])
                st[bt]['o2p'] = o2p
                st[bt]['o2rem'] = o2rem

            def phase_D(bt):
                # inverse DFT + residual, in place: out1 = hx += spat
                # psum = 1024*spat (32x in bcp/brem, 32x in the shrunk modes)
                o2p = st[bt]['o2p']
                o2rem = st[bt]['o2rem']
                hx = st[bt]['hx']
                for p in range(8):
                    for n in range(2):
                        ps = pp.tile([128, 384], f32, tag="big", bufs=4)
                        for c in range(2):
                            nc.tensor.matmul(ps[:], bcpv[:, c, :, p * 128:(p + 1) * 128],
                                             o2p[c][:, :, n * 384:(n + 1) * 384],
                                             start=(c == 0), stop=False, perf_mode=DR)
                        nc.tensor.matmul(ps[:], bremv[:, :, p * 128:(p + 1) * 128],
                                         o2rem[:, :, n * 384:(n + 1) * 384],
                                         start=False, stop=True, perf_mode=DR)
                        # keep vector entirely free for LN2 stats: scalar
                        # drains the psum, gpsimd folds it into hx
                        sp = p2.tile([128, 384], bf16, tag="dsp")
                        nc.scalar.activation(sp[:], ps[:], AF.Copy,
                                             scale=1.0 / 1024.0)
                        nc.gpsimd.tensor_tensor(
                            hx[p][:, n * 384:(n + 1) * 384], sp[:],
                            hx[p][:, n * 384:(n + 1) * 384], op=ALU.add)
                st[bt]['out1'] = hx

            def phase_E(bt):
                # LN2 -> h2 (normalized token-major bf16; affine folded into mw1/mb1)
                h2bf, _, _ = layernorm([t[:] for t in st[bt]['out1']], p8, "h2",
                                       bt=bt, nameprefix="h2")
                st[bt]['h2'] = h2bf

            def phase_F(bt, h):
                # MLP half: transpose h2 -> fp8 channel-major pairs, fp8 DoubleRow
                # 768->3072 gelu ->768, + res2, one 4-chunk DMA out
                h2bf = st[bt]['h2']
                out1 = st[bt]['out1']
                x2h = [px2.tile([128, 2, 512], fp8, tag="x2q", bufs=6,
                                name=f"x2h{bt}_{h}_{q}") for q in range(3)]
                for tcn in range(4):
                    p = h * 4 + tcn
                    for j in range(6):
                        pt = pp.tile([128, 128], bf16, tag="tpm")
                        nc.tensor.transpose(pt[:], h2bf[p][:, j * 128:(j + 1) * 128],
                                            ident_t[:])
                        nc.vector.tensor_copy(
                            x2h[j // 2][:, j % 2, tcn * 128:(tcn + 1) * 128], pt[:])
                hid = [phid.tile([128, 2, 512], fp8, tag="hid", bufs=24,
                                 name=f"hid{bt}_{h}_{qq}") for qq in range(12)]
                for fj in range(24):
                    ph = pp.tile([128, 512], f32, tag="big", bufs=4)
                    for q in range(3):
                        nc.tensor.matmul(ph[:], mw1_c(q, fj), x2h[q][:],
                                         start=(q == 0), stop=(q == 2),
                                         perf_mode=DR)
                    nc.scalar.activation(hid[fj // 2][:, fj % 2, :], ph[:],
                                         AF.Gelu, bias=mb1_c(fj), scale=1.0 / S1)
                for tcn in range(4):
                    p = h * 4 + tcn
                    ost = p8.tile([128, E], bf16, tag="xin", bufs=16,
                                  name=f"ost{bt}_{h}_{tcn}")
                    for n in range(2):
                        po = pp.tile([128, 384], f32, tag="big", bufs=4)
                        for qq in range(12):
                            nc.tensor.matmul(po[:],
                                             hid[qq][:, :, tcn * 128:(tcn + 1) * 128],
                                             mw2_c(qq, n),
                                             start=(qq == 0), stop=(qq == 11),
                                             perf_mode=DR)
                        nc.vector.scalar_tensor_tensor(
                            ost[:, n * 384:(n + 1) * 384], po[:], 1.0 / S2,
                            out1[p][:, n * 384:(n + 1) * 384],
                            op0=ALU.mult, op1=ALU.add)
                    if not mb2_zero:
                        nc.vector.tensor_add(ost[:], ost[:], mb2rep_t[:])
                    c0 = (bt * 8 + p) * E
                    oeng = (nc.sync, nc.scalar, nc.gpsimd)[p % 3]
                    oeng2 = (nc.scalar, nc.gpsimd, nc.sync)[p % 3]
                    oeng.dma_start(out_d[:, c0:c0 + E // 2], ost[:, 0:E // 2])
                    oeng2.dma_start(out_d[:, c0 + E // 2:c0 + E], ost[:, E // 2:E])

            phase_A(0)
            phase_A(1)
            fi_copy = phase_B(0)
            # deferred weight loads: don't let these race the startup burst
            # (x tiles + DFT matrices) on the HBM wire; wmix/bmix first (needed
            # by phase C right after B1)
            for dd_d, dd_t in ((wmix_d, wmix_all), (bmix_d, bmix_all),
                               (bcp_d, bcp_all), (brem_d, brem_all),
                               (mw1_d, mw1_all), (mw2_d, mw2_all),
                               (mb1_d, mb1_all)):
                dd = nc.gpsimd.dma_start(dd_t[:], dd_d[:])
                add_dep_helper(dd.ins, fi_copy.ins,
                               reason="defer bulk weight DMA past fwd DFT")
            phase_B(1)
            emit_hx(0)
            emit_hx(1)
            phase_C(0)
            phase_C(1)
            phase_D(0)
            phase_E(0)
            phase_D(1)
            phase_E(1)
            phase_F(0, 0)
            phase_F(1, 0)
            phase_F(0, 1)
            phase_F(1, 1)

    nc.compile()
    return nc


LAST_EXEC_NS = None


def make_consts(w1, b1, w2, b2, ln1_g, ln1_b, ln2_g, ln2_b,
                mw1, mb1, mw2, mb2, ln1_trivial, mb2_zero):
    art, ait, brt, bit = _dft_matrices()

    # fold ln1_g into w1 (left-diag per block over the i axis)
    g_blocks = ln1_g.reshape(NB, BS)
    W1R = _pack_blockdiag(w1[0] * g_blocks[:, :, None])
    W1I = _pack_blockdiag(w1[1] * g_blocks[:, :, None])
    W2R = _pack_blockdiag(w2[0])
    W2I = _pack_blockdiag(w2[1])

    b1r = b1[0].reshape(E)
    b1i = b1[1].reshape(E)
    b2r = b2[0].reshape(E)
    b2i = b2[1].reshape(E)

    mw1f = mw1 * ln2_g[:, None]
    mb1f = (mb1 + ln2_b @ mw1).reshape(H4)

    def bf(a):
        return np.ascontiguousarray(a.astype(_BF16))

    def fp8_pairs(a, scale):
        """(2q*128, F) -> (128, q*2*F) k-pair-interleaved fp8 image for DoubleRow."""
        nq = a.shape[0] // 256
        img = (a * scale).reshape(nq, 2, 128, a.shape[1]) \
            .transpose(2, 0, 1, 3).reshape(128, 2 * nq * a.shape[1])
        return np.ascontiguousarray(img.astype(_FP8))

    # wmix image: (128, 6 mats * 6 blocks * 128), order w1r w1i w1in w2r w2i w2in
    # w1 carries the 1/32 that was removed from the fp8 DFT matrices
    mats = [W1R / 32.0, W1I / 32.0, -W1I / 32.0, W2R, W2I, -W2I]
    wmix = np.concatenate(
        [m.transpose(1, 0, 2).reshape(128, 6 * 128) for m in mats], axis=1)
    # bmix image: (128, 36): 6 vectors x 6 chunks; shrink biases carry the
    # 32x fp8-friendly scale on the shrunk modes (undone by 1/1024 after iDFT)
    bvecs = [b1r, b1i, 32.0 * (b2r - LAM), 32.0 * (-b2r - LAM),
             32.0 * (b2i - LAM), 32.0 * (-b2i - LAM)]
    bmix = np.concatenate([v.reshape(6, 128).T for v in bvecs], axis=1)

    # iDFT pair image: [128, c(2), k(2), 1024] slot0=brt, slot1=bit (32x scale)
    brt32 = brt * 32.0
    bit32 = bit * 32.0
    bcp = np.zeros((128, 2, 2, NTOK), np.float32)
    for c in range(2):
        bcp[:, c, 0, :] = brt32[c * 128:(c + 1) * 128]
        bcp[:, c, 1, :] = bit32[c * 128:(c + 1) * 128]
    # remainder modes 256:288 as a zero-padded DoubleRow pair (r parts 0:32,
    # i parts 32:64, slot1 all zero)
    brem = np.zeros((128, 2, NTOK), np.float32)
    brem[0:32, 0, :] = brt32[256:288]
    brem[32:64, 0, :] = bit32[256:288]

    consts = {
        "art": fp8_pairs(art, 32.0), "ait": fp8_pairs(ait, 32.0),
        "bcp": np.ascontiguousarray(bcp.reshape(128, 4 * NTOK).astype(_FP8)),
        "brem": np.ascontiguousarray(brem.reshape(128, 2 * NTOK).astype(_FP8)),
        "wmix": bf(wmix), "bmix": np.ascontiguousarray(bmix, np.float32),
        "mw1f": fp8_pairs(mw1f, S1),
        "mb1f": np.ascontiguousarray(mb1f.reshape(24, 128).T, np.float32),
        "mw2": fp8_pairs(mw2, S2),
        "ident": bf(np.eye(128, dtype=np.float32)),
    }
    if not ln1_trivial:
        consts["g1rep"] = np.tile(ln1_g[None, :], (128, 1)).astype(np.float32)
        consts["b1rep"] = np.tile(ln1_b[None, :], (128, 1)).astype(np.float32)
    if not mb2_zero:
        consts["mb2rep"] = np.tile(mb2[None, :], (128, 1)).astype(np.float32)
    return consts


def kernel(input, w1, b1, w2, b2, ln1_g, ln1_b, ln2_g, ln2_b, mw1, mb1, mw2, mb2):
    global LAST_EXEC_NS
    _install_trace_shim()
    import os
    from concourse.bass_utils import run_bass_kernel_spmd

    input = np.asarray(input, np.float32)
    w1 = np.asarray(w1, np.float32)
    b1 = np.asarray(b1, np.float32)
    w2 = np.asarray(w2, np.float32)
    b2 = np.asarray(b2, np.float32)
    ln1_g = np.asarray(ln1_g, np.float32)
    ln1_b = np.asarray(ln1_b, np.float32)
    ln2_g = np.asarray(ln2_g, np.float32)
    ln2_b = np.asarray(ln2_b, np.float32)
    mw1 = np.asarray(mw1, np.float32)
    mb1 = np.asarray(mb1, np.float32)
    mw2 = np.asarray(mw2, np.float32)
    mb2 = np.asarray(mb2, np.float32)

    ln1_trivial = bool(np.all(ln1_g == 1.0) and np.all(ln1_b == 0.0))
    mb2_zero = bool(np.all(mb2 == 0.0))

    key = (ln1_trivial, mb2_zero)
    if key not in _CACHE:
        _CACHE[key] = _build_program(ln1_trivial, mb2_zero)
    nc = _CACHE[key]

    consts = make_consts(w1, b1, w2, b2, ln1_g, ln1_b, ln2_g, ln2_b,
                         mw1, mb1, mw2, mb2, ln1_trivial, mb2_zero)

    xs = input.reshape(B * T, NTOK, E)
    in_maps = []
    for c in range(NCORES):
        shard = xs[c * BT_PER_CORE:(c + 1) * BT_PER_CORE].reshape(TOK_CORE, E)
        # partition-major image: img[p, k*E:(k+1)*E] = x[k*128+p, :]
        img = np.ascontiguousarray(
            shard.reshape(16, 128, E).transpose(1, 0, 2).reshape(128, 16 * E)
            .astype(_BF16))
        m = {"x": img}
        m.update(consts)
        in_maps.append(m)

    trace = bool(os.environ.get("BASS_TRACE"))
    res = run_bass_kernel_spmd(nc, in_maps, core_ids=list(range(NCORES)),
                               trace=trace)
    LAST_EXEC_NS = res.exec_time_ns
    out = np.concatenate(
        [np.asarray(res.results[c]["out"]).astype(np.float32)
         .reshape(128, 16, E).transpose(1, 0, 2)
         .reshape(BT_PER_CORE, NTOK, E) for c in range(NCORES)], axis=0)
    return out.reshape(B, T, NTOK, E)

